# revision 20
# baseline (speedup 1.0000x reference)
"""Trainium2 Bass kernel for nn_DirectPoseOutputs (loss_fn).

Reference computation:
  1) 3x3 max-pool NMS on heat [16,17,100,152]
  2) per-channel top-40 (scores + flat indices), jax.lax.top_k tie order
  3) gather feat [16,256,15200] columns at the top-40 indices -> [B,680,256]
  4) ctrness mean over reg_targets [20000,4] (scalar)
  out = gathered * scores + mean_ctrness

Sharding: data-parallel over batch, 2 images per core, 8 cores. ctrness is
computed redundantly on every core (320KB read, no collectives).

Per-core pipeline (engine assignment in brackets):
  feat stream [SP queue only]: 4x [128,15200] tiles, ~21us each.
  heat path: DRAM->DRAM transpose to y-major [ACT], load ht + y-shifted
  copies [ACT]; separable NMS: H-max from shifted loads + W-max via free-dim
  shifts [DVE], mask (is_ge, mult) on [GPSIMD]; stage-1 per-y-row top-8
  via InstMax/InstMaxIndex [DVE]; stage-2 relayout to per-channel rows via
  PE transpose + coarse-grained fold DMA (m = s*100+y order; validated safe
  for the duplicate-value tie cases); stage-3 top-40 via 5 rounds of
  max/max_index/match_replace [DVE]; stage-4 global indices via 6 indirect
  DMAs of [128,1] (one offset per dest partition row -- HW semantics);
  feat gather via GPSIMD indirect_copy (shared wrapped index table);
  PE-transpose [c,j]->[j,c]; fused out = psum*score[j] + mean [ACT]; out
  DMAs [SP].
"""
import numpy as np

import concourse.bass as bass
import concourse.bacc as bacc
import concourse.mybir as mybir
from concourse.bass_types import AP
from concourse.bass_utils import run_bass_kernel_spmd
from concourse.masks import make_identity
from concourse.tile import TileContext

F32 = mybir.dt.float32
U16 = mybir.dt.uint16
U32 = mybir.dt.uint32

B, CK, H, W = 16, 17, 100, 152
HW = H * W            # 15200
CW = CK * W           # 2584 free width of the y-major heat tile
CF = 256
K = 40
NCORES = 8
IPC = B // NCORES     # images per core = 2
J = CK * K            # 680 output rows per image
JP = 768              # padded to 6 tiles of 128 rows (and %16 for the table)
NREG = 20000
MAX, GE, MUL, ADD = (mybir.AluOpType.max, mybir.AluOpType.is_ge,
                     mybir.AluOpType.mult, mybir.AluOpType.add)


def build_program() -> bass.Bass:
    nc = bacc.Bacc()

    heat_in = nc.declare_dram_parameter("heat", [IPC, CK, H, W], F32, isOutput=False)
    feat_in = nc.declare_dram_parameter("feat", [IPC, CF, HW], F32, isOutput=False)
    regs_in = nc.declare_dram_parameter("regs", [NREG, 4], F32, isOutput=False)
    out_o = nc.declare_dram_parameter("out", [IPC, J, CF], F32, isOutput=True)

    # constants
    rowbase_c = nc.inline_tensor(
        np.broadcast_to((np.arange(H, dtype=np.float32) * W)[:, None],
                        (H, CK * 8)).copy(), "rowbasec")
    chbase_c = nc.inline_tensor(
        np.broadcast_to((np.arange(CK, dtype=np.uint32) * 800)[:, None],
                        (CK, K)).copy(), "chbasec")
    zpad32_c = nc.inline_tensor(np.zeros((1, JP - J), np.uint32), "zpad32c")
    fpad_c = nc.inline_tensor(np.zeros((1, JP - J), np.float32), "fpadc")

    # DRAM scratch
    ht_d = nc.dram_tensor("ht_d", [IPC, H, CK, W], F32)      # y-major heat
    cd_d = [nc.dram_tensor(f"cd_d{i}", [136, H], F32) for i in range(IPC)]
    gd_d = [nc.dram_tensor(f"gd_d{i}", [136, H], F32) for i in range(IPC)]
    g16_d = [nc.dram_tensor(f"g16_d{i}", [CK * 800], U16) for i in range(IPC)]
    sc_d = nc.dram_tensor("sc_d", [IPC, JP], F32)            # scores bounce
    si_d = nc.dram_tensor("si_d", [IPC, JP], U16)            # gather table bounce
    so_d = nc.dram_tensor("so_d", [IPC, JP], U32)            # stage-4 offsets

    with TileContext(nc) as tc:
        with (
            tc.tile_pool(name="consts", bufs=1) as cp,
            tc.tile_pool(name="pers", bufs=1) as pp,
            tc.tile_pool(name="heat", bufs=1) as hp,
            tc.tile_pool(name="ft", bufs=2) as fp,
            tc.tile_pool(name="gt", bufs=2) as gp,
            tc.tile_pool(name="osb", bufs=4) as op,
            tc.tile_pool(name="ps", bufs=1, space="PSUM") as qp,
            tc.tile_pool(name="psc", bufs=1, space="PSUM") as qcp,
        ):
            # ---- feat stream + heat prefetches start immediately ---------
            fts = {}
            for im in range(IPC):
                for t in range(2):
                    ft = fp.tile([128, HW], F32, tag="ft", name=f"ft{im}_{t}")
                    if im == 0:
                        nc.sync.dma_start(out=ft,
                                          in_=feat_in[im, 128 * t:128 * (t + 1), :])
                    fts[(im, t)] = ft
            for im in range(IPC):
                nc.scalar.dma_start(
                    out=ht_d[im].rearrange("h c w -> c h w"),
                    in_=heat_in[im])

            # ---------------- ctrness mean --------------------------------
            rt = cp.tile([125, 160, 4], F32, tag="regs")
            nc.scalar.dma_start(out=rt,
                                in_=regs_in[:].rearrange("(p a) b -> p a b", p=125))
            l_, t_, r_, b_ = (rt[:, :, i] for i in range(4))
            mn = cp.tile([125, 160], F32, tag="ctr_mn")
            mx = cp.tile([125, 160], F32, tag="ctr_mx")
            pr = cp.tile([125, 160], F32, tag="ctr_pr")
            nc.vector.tensor_tensor(out=mn, in0=l_, in1=r_, op=mybir.AluOpType.min)
            nc.vector.tensor_tensor(out=mx, in0=l_, in1=r_, op=MAX)
            nc.vector.reciprocal(out=mx, in_=mx)
            nc.vector.tensor_tensor(out=pr, in0=mn, in1=mx, op=MUL)
            nc.vector.tensor_tensor(out=mn, in0=t_, in1=b_, op=mybir.AluOpType.min)
            nc.vector.tensor_tensor(out=mx, in0=t_, in1=b_, op=MAX)
            nc.vector.reciprocal(out=mx, in_=mx)
            nc.vector.tensor_tensor(out=mn, in0=mn, in1=mx, op=MUL)
            nc.vector.tensor_tensor(out=pr, in0=pr, in1=mn, op=MUL)
            acc = cp.tile([128, 1], F32, tag="ctr_acc")
            nc.vector.memset(acc, 0.0)
            ctr_s = cp.tile([125, 160], F32, tag="ctr_s")
            nc.scalar.activation(out=ctr_s, in_=pr,
                                 func=mybir.ActivationFunctionType.Sqrt,
                                 accum_out=acc[0:125, :])
            ones_col = cp.tile([128, 1], F32, tag="ones_col")
            nc.vector.memset(ones_col, 1.0)
            ones_row = cp.tile([1, 128], F32, tag="ones_row")
            nc.vector.memset(ones_row, 1.0)
            tot_p = qcp.tile([1, 1], F32, tag="aux0")
            nc.tensor.matmul(tot_p, ones_col, acc, start=True, stop=True)
            mean1 = cp.tile([1, 1], F32, tag="mean1")
            nc.scalar.activation(out=mean1, in_=tot_p,
                                 func=mybir.ActivationFunctionType.Copy,
                                 scale=1.0 / NREG)
            mean_p = qcp.tile([128, 1], F32, tag="aux1")
            nc.tensor.matmul(mean_p, ones_row, mean1, start=True, stop=True)
            mean128 = cp.tile([128, 1], F32, tag="mean128")
            nc.vector.tensor_copy(out=mean128, in_=mean_p)

            ident = cp.tile([128, 128], F32, tag="ident")
            make_identity(nc, ident)
            rowbase = cp.tile([H, CK, 8], F32, tag="rowbase")
            nc.scalar.dma_start(out=rowbase, in_=rowbase_c[:].rearrange(
                "h (c s) -> h c s", s=8))
            chbase = cp.tile([CK, K], U32, tag="chbase")
            nc.scalar.dma_start(out=chbase, in_=chbase_c[:])

            score_sb = []
            widx = []
            # ---------------- heat pipeline, one image at a time ----------
            for im in range(IPC):
                ht = hp.tile([H, CK, W], F32, tag="ht")
                hu = hp.tile([H, CK, W], F32, tag="hu")
                hd = hp.tile([H, CK, W], F32, tag="hd")
                nc.scalar.dma_start(out=ht, in_=ht_d[im])
                # y-shifted copies with clamped edges (prefetchable from DRAM)
                nc.scalar.dma_start(out=hu[0:H - 1], in_=ht_d[im, 1:H])
                nc.scalar.dma_start(out=hu[H - 1:H], in_=ht_d[im, H - 1:H])
                nc.scalar.dma_start(out=hd[1:H], in_=ht_d[im, 0:H - 1])
                nc.scalar.dma_start(out=hd[0:1], in_=ht_d[im, 0:1])
                # H-direction 3-max into hu
                nc.vector.tensor_tensor(out=hu, in0=hu, in1=ht, op=MAX)
                nc.vector.tensor_tensor(out=hu, in0=hu, in1=hd, op=MAX)
                # W-direction 3-max into hd
                nc.vector.tensor_tensor(out=hd[:, :, 0:W - 1], in0=hu[:, :, 0:W - 1],
                                        in1=hu[:, :, 1:W], op=MAX)
                nc.vector.tensor_copy(out=hd[:, :, W - 1:W], in_=hu[:, :, W - 1:W])
                nc.vector.tensor_tensor(out=hd[:, :, 1:W], in0=hd[:, :, 1:W],
                                        in1=hu[:, :, 0:W - 1], op=MAX)
                # keep only local maxima: ht *= (ht >= hd)
                nc.vector.tensor_tensor(out=hu, in0=ht, in1=hd, op=GE)
                nc.gpsimd.tensor_tensor(out=ht, in0=ht, in1=hu, op=MUL)

                # stage 1: per-y-row top-8 values + row-local indices
                vals8 = pp.tile([H, CK, 8], F32, tag=f"vals8_{im}")
                rl8 = pp.tile([H, CK, 8], U16, tag=f"rl8_{im}")
                for ch in range(CK):
                    nc.vector.max(out=vals8[:, ch, :], in_=ht[:, ch, :])
                    nc.vector.max_index(out=rl8[:, ch, :], in_max=vals8[:, ch, :],
                                        in_values=ht[:, ch, :])
                # global flat index, in f32 (exact below 2^24)
                gidxf = pp.tile([H, CK, 8], F32, tag=f"gidxf_{im}")
                nc.vector.tensor_copy(out=gidxf, in_=rl8)
                nc.vector.tensor_tensor(out=gidxf, in0=gidxf, in1=rowbase, op=ADD)

                # stage 2: transpose candidates to per-channel rows via PE.
                # m-order: m = s*100 + y  (cand[ch, m] = vals8[y, ch, s])
                cand = pp.tile([CK, 800], F32, tag=f"cand_{im}")
                g16f = pp.tile([CK, 800], F32, tag=f"g16f_{im}")
                for srct, dst, dscr in ((vals8, cand, cd_d[im]),
                                        (gidxf, g16f, gd_d[im])):
                    for blk in range(2):
                        p0, pn = blk * 128, min(136 - blk * 128, 128)
                        tp = qcp.tile([128, H], F32, tag=f"aux{blk}",
                                      name=f"st2_{im}_{blk}_{dst.tensor.name}")
                        nc.tensor.transpose(
                            out=tp[0:pn, :],
                            in_=srct[:].rearrange("h c s -> h (c s)")[:, p0:p0 + pn],
                            identity=ident[0:H, 0:H])
                        tps = pp.tile([128, H], F32, tag=f"st2s_{blk}",
                                      name=f"st2s_{im}_{blk}_{dst.tensor.name}")
                        nc.vector.tensor_copy(out=tps[0:pn, :], in_=tp[0:pn, :])
                        nc.scalar.dma_start(out=dscr[p0:p0 + pn, :], in_=tps[0:pn, :])
                    # dst[ch, s*100+y] = dscr[ch*8+s, y]
                    nc.scalar.dma_start(
                        out=dst,
                        in_=dscr[:].rearrange("(c s) h -> c (s h)", s=8))

                # stage 3: per-channel top-40 of the 800 candidates
                tv = pp.tile([CK, K], F32, tag=f"tv_{im}")
                cpos = pp.tile([CK, K], U32, tag=f"cpos_{im}")
                for r in range(5):
                    v8 = tv[:, r * 8:(r + 1) * 8]
                    nc.vector.max(out=v8, in_=cand)
                    nc.vector.max_index(out=cpos[:, r * 8:(r + 1) * 8], in_max=v8,
                                        in_values=cand)
                    nc.vector.match_replace(out=cand, in_to_replace=v8,
                                            in_values=cand, imm_value=-1.0)

                # stage 4: global indices.  g16f (f32) -> u16 -> DRAM
                # (contiguous), then 6 indirect row-gathers of [128, 1].
                g16u = pp.tile([CK, 800], U16, tag=f"g16u_{im}")
                nc.vector.tensor_copy(out=g16u, in_=g16f)
                nc.scalar.dma_start(out=g16_d[im][:].rearrange("(c m) -> c m", m=800),
                                    in_=g16u)
                offs = pp.tile([CK, K], U32, tag=f"offs_{im}")
                nc.vector.tensor_tensor(out=offs, in0=cpos, in1=chbase, op=ADD)
                nc.scalar.dma_start(out=so_d[im, 0:J].rearrange("(c k) -> c k", k=K),
                                    in_=offs)
                nc.scalar.dma_start(out=so_d[im, J:JP], in_=zpad32_c[0])
                o128 = pp.tile([128, JP // 128], U32, tag=f"o128_{im}")
                nc.scalar.dma_start(
                    out=o128,
                    in_=AP(tensor=so_d[:].tensor, offset=im * JP,
                           ap=[[1, 128], [128, JP // 128]]))
                gi128 = pp.tile([128, JP // 128], U16, tag=f"gi128_{im}")
                for q in range(JP // 128):
                    nc.gpsimd.indirect_dma_start(
                        out=gi128[:, q:q + 1], out_offset=None,
                        in_=AP(tensor=g16_d[im][:].tensor, offset=0,
                               ap=[[1, CK * 800], [1, 1]]),
                        in_offset=bass.IndirectOffsetOnAxis(ap=o128[:, q:q + 1],
                                                            axis=0))

                # stage 5: wrapped gather table + per-tile scores
                nc.scalar.dma_start(
                    out=AP(tensor=si_d[:].tensor, offset=im * JP,
                           ap=[[1, 128], [128, JP // 128]]),
                    in_=gi128)
                wi = pp.tile([128, JP // 16], U16, tag=f"widx_{im}")
                for grp in range(8):
                    nc.scalar.dma_start(
                        out=wi[16 * grp:16 * (grp + 1), :],
                        in_=AP(tensor=si_d[:].tensor, offset=im * JP,
                               ap=[[1, 16], [16, JP // 16]]))
                widx.append(wi)
                nc.scalar.dma_start(out=sc_d[im, 0:J].rearrange("(c k) -> c k", k=K),
                                    in_=tv)
                nc.scalar.dma_start(out=sc_d[im, J:JP], in_=fpad_c[0])
                ssb = pp.tile([128, JP // 128], F32, tag=f"ssb_{im}")
                nc.scalar.dma_start(
                    out=ssb,
                    in_=AP(tensor=sc_d[:].tensor, offset=im * JP,
                           ap=[[1, 128], [128, JP // 128]]))
                score_sb.append(ssb)

            # ---------------- feat gather / output ------------------------
            for im in range(IPC):
                pts = []
                for q in range(6):
                    pts.append(qp.tile([128, 256], F32, tag=f"pt{q}",
                                       name=f"pt{im}_{q}"))
                for t in range(2):
                    ft = fts[(im, t)]
                    if im > 0:
                        nc.sync.dma_start(out=ft,
                                          in_=feat_in[im, 128 * t:128 * (t + 1), :])
                    g = gp.tile([128, JP], F32, tag="g", name=f"g{im}_{t}")
                    nc.gpsimd.indirect_copy(out=g, data=ft, idxs=widx[im],
                                            i_know_ap_gather_is_preferred=True)
                    for q in range(6):
                        nc.tensor.transpose(out=pts[q][:, 128 * t:128 * (t + 1)],
                                            in_=g[:, 128 * q:128 * (q + 1)],
                                            identity=ident)
                for q in range(6):
                    osb = op.tile([128, 256], F32, tag="osb", name=f"osb{im}_{q}")
                    nc.scalar.activation(out=osb, in_=pts[q],
                                         func=mybir.ActivationFunctionType.Identity,
                                         bias=mean128[:, :1],
                                         scale=score_sb[im][:, q:q + 1])
                    if q < 5:
                        nc.sync.dma_start(out=out_o[im, 128 * q:128 * (q + 1), :],
                                          in_=osb)
                    else:
                        nc.sync.dma_start(out=out_o[im, 640:J, :],
                                          in_=osb[0:J - 640, :])
    nc.finalize()
    return nc


def kernel(heat: np.ndarray, feat: np.ndarray, reg_targets: np.ndarray) -> np.ndarray:
    heat = np.ascontiguousarray(heat, dtype=np.float32)
    feat = np.ascontiguousarray(feat, dtype=np.float32).reshape(B, CF, HW)
    regs = np.ascontiguousarray(reg_targets, dtype=np.float32)

    nc = build_program()
    in_maps = [
        {"heat": heat[c * IPC:(c + 1) * IPC],
         "feat": feat[c * IPC:(c + 1) * IPC],
         "regs": regs}
        for c in range(NCORES)
    ]
    res = run_bass_kernel_spmd(nc, in_maps, list(range(NCORES)))
    out = np.concatenate([np.asarray(r["out"]) for r in res.results], axis=0)
    return out.reshape(B, J, CF)


if __name__ == "__main__":
    import ref_numpy as RN
    inputs = RN.get_inputs()
    exp = RN.get_expected(inputs)
    got = kernel(inputs["heat"], inputs["feat"], inputs["reg_targets"])
    err = np.abs(got - exp).max() / np.abs(exp).max()
    print("Relative error:", err)


# revision 22
# speedup vs baseline: 1.1359x; 1.1359x over previous
"""Trainium2 Bass kernel for nn_DirectPoseOutputs (loss_fn).

Reference computation:
  1) 3x3 max-pool NMS on heat [16,17,100,152]
  2) per-channel top-40 (scores + flat indices), jax.lax.top_k tie order
  3) gather feat [16,256,15200] columns at the top-40 indices -> [B,680,256]
  4) ctrness mean over reg_targets [20000,4] (scalar)
  out = gathered * scores + mean_ctrness

Sharding: data-parallel over batch, 2 images per core, 8 cores. ctrness is
computed redundantly on every core (320KB read, no collectives).

Per-core pipeline (engine assignment in brackets):
  feat stream [SP queue only]: 4x [128,15200] tiles, ~21us each.
  heat path: DRAM->DRAM transpose to y-major [ACT], load ht + y-shifted
  copies [ACT]; separable NMS: H-max from shifted loads + W-max via free-dim
  shifts [DVE], mask (is_ge, mult) on [GPSIMD]; stage-1 per-y-row top-8
  via InstMax/InstMaxIndex [DVE]; stage-2 relayout to per-channel rows via
  PE transpose + coarse-grained fold DMA (m = s*100+y order; validated safe
  for the duplicate-value tie cases); stage-3 top-40 via 5 rounds of
  max/max_index/match_replace [DVE]; stage-4 global indices via 6 indirect
  DMAs of [128,1] (one offset per dest partition row -- HW semantics);
  feat gather via GPSIMD indirect_copy (shared wrapped index table);
  PE-transpose [c,j]->[j,c]; fused out = psum*score[j] + mean [ACT]; out
  DMAs [SP].
"""
import numpy as np

import concourse.bass as bass
import concourse.bacc as bacc
import concourse.mybir as mybir
from concourse.bass_types import AP
from concourse.bass_utils import run_bass_kernel_spmd
from concourse.masks import make_identity
from concourse.tile import TileContext

F32 = mybir.dt.float32
U16 = mybir.dt.uint16
U32 = mybir.dt.uint32

B, CK, H, W = 16, 17, 100, 152
HW = H * W            # 15200
CW = CK * W           # 2584 free width of the y-major heat tile
CF = 256
K = 40
NCORES = 8
IPC = B // NCORES     # images per core = 2
J = CK * K            # 680 output rows per image
JP = 768              # padded to 6 tiles of 128 rows (and %16 for the table)
NREG = 20000
MAX, GE, MUL, ADD = (mybir.AluOpType.max, mybir.AluOpType.is_ge,
                     mybir.AluOpType.mult, mybir.AluOpType.add)


def build_program() -> bass.Bass:
    nc = bacc.Bacc()

    heat_in = nc.declare_dram_parameter("heat", [IPC, CK, H, W], F32, isOutput=False)
    feat_in = nc.declare_dram_parameter("feat", [IPC, CF, HW], F32, isOutput=False)
    regs_in = nc.declare_dram_parameter("regs", [NREG, 4], F32, isOutput=False)
    out_o = nc.declare_dram_parameter("out", [IPC, J, CF], F32, isOutput=True)

    # constants
    rowbase_c = nc.inline_tensor(
        np.broadcast_to((np.arange(H, dtype=np.float32) * W)[:, None],
                        (H, CK * 8)).copy(), "rowbasec")
    chbase_c = nc.inline_tensor(
        np.broadcast_to((np.arange(CK, dtype=np.uint32) * 800)[:, None],
                        (CK, K)).copy(), "chbasec")
    zpad32_c = nc.inline_tensor(np.zeros((1, JP - J), np.uint32), "zpad32c")
    fpad_c = nc.inline_tensor(np.zeros((1, JP - J), np.float32), "fpadc")

    # DRAM scratch
    cd_d = [nc.dram_tensor(f"cd_d{i}", [136, H], F32) for i in range(IPC)]
    gd_d = [nc.dram_tensor(f"gd_d{i}", [136, H], F32) for i in range(IPC)]
    g16_d = [nc.dram_tensor(f"g16_d{i}", [CK * 800], U16) for i in range(IPC)]
    sc_d = nc.dram_tensor("sc_d", [IPC, JP], F32)            # scores bounce
    si_d = nc.dram_tensor("si_d", [IPC, JP], U16)            # gather table bounce
    so_d = nc.dram_tensor("so_d", [IPC, JP], U32)            # stage-4 offsets

    with TileContext(nc) as tc:
        with (
            tc.tile_pool(name="consts", bufs=1) as cp,
            tc.tile_pool(name="pers", bufs=1) as pp,
            tc.tile_pool(name="heat", bufs=1) as hp,
            tc.tile_pool(name="ft", bufs=2) as fp,
            tc.tile_pool(name="gt", bufs=2) as gp,
            tc.tile_pool(name="osb", bufs=4) as op,
            tc.tile_pool(name="ps", bufs=1, space="PSUM") as qp,
            tc.tile_pool(name="psc", bufs=1, space="PSUM") as qcp,
        ):
            # ---- feat stream + heat prefetches start immediately ---------
            fts = {}
            for im in range(IPC):
                for t in range(2):
                    ft = fp.tile([128, HW], F32, tag="ft", name=f"ft{im}_{t}")
                    if im == 0:
                        nc.sync.dma_start(out=ft,
                                          in_=feat_in[im, 128 * t:128 * (t + 1), :])
                    fts[(im, t)] = ft
            def emit_ctr():
                rt = cp.tile([125, 160, 4], F32, tag="regs", name="rt")
                nc.sync.dma_start(out=rt,
                                  in_=regs_in[:].rearrange("(p a) b -> p a b", p=125))
                l_, t_, r_, b_ = (rt[:, :, i] for i in range(4))
                mn = cp.tile([125, 160], F32, tag="ctr_mn", name="ctr_mn")
                mx = cp.tile([125, 160], F32, tag="ctr_mx", name="ctr_mx")
                pr = cp.tile([125, 160], F32, tag="ctr_pr", name="ctr_pr")
                nc.vector.tensor_tensor(out=mn, in0=l_, in1=r_, op=mybir.AluOpType.min)
                nc.vector.tensor_tensor(out=mx, in0=l_, in1=r_, op=MAX)
                nc.vector.reciprocal(out=mx, in_=mx)
                nc.vector.tensor_tensor(out=pr, in0=mn, in1=mx, op=MUL)
                nc.vector.tensor_tensor(out=mn, in0=t_, in1=b_, op=mybir.AluOpType.min)
                nc.vector.tensor_tensor(out=mx, in0=t_, in1=b_, op=MAX)
                nc.vector.reciprocal(out=mx, in_=mx)
                nc.vector.tensor_tensor(out=mn, in0=mn, in1=mx, op=MUL)
                nc.vector.tensor_tensor(out=pr, in0=pr, in1=mn, op=MUL)
                acc = cp.tile([128, 1], F32, tag="ctr_acc", name="ctr_acc")
                nc.vector.memset(acc, 0.0)
                ctr_s = cp.tile([125, 160], F32, tag="ctr_s", name="ctr_s")
                nc.scalar.activation(out=ctr_s, in_=pr,
                                     func=mybir.ActivationFunctionType.Sqrt,
                                     accum_out=acc[0:125, :])
                ones_col = cp.tile([128, 1], F32, tag="ones_col", name="ones_col")
                nc.vector.memset(ones_col, 1.0)
                ones_row = cp.tile([1, 128], F32, tag="ones_row", name="ones_row")
                nc.vector.memset(ones_row, 1.0)
                tot_p = qcp.tile([1, 1], F32, tag="aux0", name="tot_p")
                nc.tensor.matmul(tot_p, ones_col, acc, start=True, stop=True)
                mean1 = cp.tile([1, 1], F32, tag="mean1", name="mean1")
                nc.scalar.activation(out=mean1, in_=tot_p,
                                     func=mybir.ActivationFunctionType.Copy,
                                     scale=1.0 / NREG)
                mean_p = qcp.tile([128, 1], F32, tag="aux1", name="mean_p")
                nc.tensor.matmul(mean_p, ones_row, mean1, start=True, stop=True)
                mean128 = cp.tile([128, 1], F32, tag="mean128", name="mean128")
                nc.vector.tensor_copy(out=mean128, in_=mean_p)
                return mean128

            ident = cp.tile([128, 128], F32, tag="ident")
            make_identity(nc, ident)
            rowbase = cp.tile([H, CK, 8], F32, tag="rowbase")
            nc.sync.dma_start(out=rowbase, in_=rowbase_c[:].rearrange(
                "h (c s) -> h c s", s=8))
            chbase = cp.tile([CK, K], U32, tag="chbase")
            nc.sync.dma_start(out=chbase, in_=chbase_c[:])
            mean128 = None

            score_sb = []
            widx = []
            # ---------------- heat pipeline, one image at a time ----------
            for im in range(IPC):
                ldq = nc.scalar if im == 0 else nc.sync
                shq = nc.gpsimd if im == 0 else nc.sync
                ht = hp.tile([H, CK, W], F32, tag="ht")
                hu = hp.tile([H, CK, W], F32, tag="hu")
                hd = hp.tile([H, CK, W], F32, tag="hd")
                for ch in range(CK):
                    ldq.dma_start(out=ht[:, ch, :], in_=heat_in[im, ch])
                # y-shifted copies with clamped edges (SBUF->SBUF)
                shq.dma_start(out=hu[0:H - 1], in_=ht[1:H])
                shq.dma_start(out=hu[H - 1:H], in_=ht[H - 1:H])
                shq.dma_start(out=hd[1:H], in_=ht[0:H - 1])
                shq.dma_start(out=hd[0:1], in_=ht[0:1])
                # H-direction 3-max into hu
                nc.vector.tensor_tensor(out=hu, in0=hu, in1=ht, op=MAX)
                nc.vector.tensor_tensor(out=hu, in0=hu, in1=hd, op=MAX)
                # W-direction 3-max into hd
                nc.vector.tensor_tensor(out=hd[:, :, 0:W - 1], in0=hu[:, :, 0:W - 1],
                                        in1=hu[:, :, 1:W], op=MAX)
                nc.vector.tensor_copy(out=hd[:, :, W - 1:W], in_=hu[:, :, W - 1:W])
                nc.vector.tensor_tensor(out=hd[:, :, 1:W], in0=hd[:, :, 1:W],
                                        in1=hu[:, :, 0:W - 1], op=MAX)
                # keep only local maxima: ht *= (ht >= hd)
                nc.vector.tensor_tensor(out=hu, in0=ht, in1=hd, op=GE)
                nc.gpsimd.tensor_tensor(out=ht, in0=ht, in1=hu, op=MUL)

                # stage 1: per-y-row top-8 values + row-local indices
                vals8 = pp.tile([H, CK, 8], F32, tag=f"vals8_{im}")
                rl8 = pp.tile([H, CK, 8], U16, tag=f"rl8_{im}")
                for ch in range(CK):
                    nc.vector.max(out=vals8[:, ch, :], in_=ht[:, ch, :])
                    nc.vector.max_index(out=rl8[:, ch, :], in_max=vals8[:, ch, :],
                                        in_values=ht[:, ch, :])
                # global flat index, in f32 (exact below 2^24)
                gidxf = pp.tile([H, CK, 8], F32, tag=f"gidxf_{im}")
                nc.vector.tensor_copy(out=gidxf, in_=rl8)
                nc.vector.tensor_tensor(out=gidxf, in0=gidxf, in1=rowbase, op=ADD)

                # stage 2: transpose candidates to per-channel rows via PE.
                # m-order: m = s*100 + y  (cand[ch, m] = vals8[y, ch, s])
                cand = pp.tile([CK, 800], F32, tag=f"cand_{im}")
                g16f = pp.tile([CK, 800], F32, tag=f"g16f_{im}")
                for srct, dst, dscr in ((vals8, cand, cd_d[im]),
                                        (gidxf, g16f, gd_d[im])):
                    for blk in range(2):
                        p0, pn = blk * 128, min(136 - blk * 128, 128)
                        tp = qcp.tile([128, H], F32, tag=f"aux{blk}",
                                      name=f"st2_{im}_{blk}_{dst.tensor.name}")
                        nc.tensor.transpose(
                            out=tp[0:pn, :],
                            in_=srct[:].rearrange("h c s -> h (c s)")[:, p0:p0 + pn],
                            identity=ident[0:H, 0:H])
                        tps = pp.tile([128, H], F32, tag=f"st2s_{blk}",
                                      name=f"st2s_{im}_{blk}_{dst.tensor.name}")
                        nc.vector.tensor_copy(out=tps[0:pn, :], in_=tp[0:pn, :])
                        nc.scalar.dma_start(out=dscr[p0:p0 + pn, :], in_=tps[0:pn, :])
                    # dst[ch, s*100+y] = dscr[ch*8+s, y]
                    nc.scalar.dma_start(
                        out=dst,
                        in_=dscr[:].rearrange("(c s) h -> c (s h)", s=8))

                # stage 3: per-channel top-40 of the 800 candidates
                tv = pp.tile([CK, K], F32, tag=f"tv_{im}")
                cpos = pp.tile([CK, K], U32, tag=f"cpos_{im}")
                for r in range(5):
                    v8 = tv[:, r * 8:(r + 1) * 8]
                    nc.vector.max(out=v8, in_=cand)
                    nc.vector.max_index(out=cpos[:, r * 8:(r + 1) * 8], in_max=v8,
                                        in_values=cand)
                    nc.vector.match_replace(out=cand, in_to_replace=v8,
                                            in_values=cand, imm_value=-1.0)

                # stage 4: global indices.  g16f (f32) -> u16 -> DRAM
                # (contiguous), then 6 indirect row-gathers of [128, 1].
                g16u = pp.tile([CK, 800], U16, tag=f"g16u_{im}")
                nc.vector.tensor_copy(out=g16u, in_=g16f)
                nc.scalar.dma_start(out=g16_d[im][:].rearrange("(c m) -> c m", m=800),
                                    in_=g16u)
                offs = pp.tile([CK, K], U32, tag=f"offs_{im}")
                nc.vector.tensor_tensor(out=offs, in0=cpos, in1=chbase, op=ADD)
                nc.scalar.dma_start(out=so_d[im, 0:J].rearrange("(c k) -> c k", k=K),
                                    in_=offs)
                nc.scalar.dma_start(out=so_d[im, J:JP], in_=zpad32_c[0])
                o128 = pp.tile([128, JP // 128], U32, tag=f"o128_{im}")
                nc.scalar.dma_start(
                    out=o128,
                    in_=AP(tensor=so_d[:].tensor, offset=im * JP,
                           ap=[[1, 128], [128, JP // 128]]))
                gi128 = pp.tile([128, JP // 128], U16, tag=f"gi128_{im}")
                for q in range(JP // 128):
                    nc.gpsimd.indirect_dma_start(
                        out=gi128[:, q:q + 1], out_offset=None,
                        in_=AP(tensor=g16_d[im][:].tensor, offset=0,
                               ap=[[1, CK * 800], [1, 1]]),
                        in_offset=bass.IndirectOffsetOnAxis(ap=o128[:, q:q + 1],
                                                            axis=0))

                # stage 5: wrapped gather table + per-tile scores
                nc.scalar.dma_start(
                    out=AP(tensor=si_d[:].tensor, offset=im * JP,
                           ap=[[1, 128], [128, JP // 128]]),
                    in_=gi128)
                wi = pp.tile([128, JP // 16], U16, tag=f"widx_{im}")
                for grp in range(8):
                    nc.scalar.dma_start(
                        out=wi[16 * grp:16 * (grp + 1), :],
                        in_=AP(tensor=si_d[:].tensor, offset=im * JP,
                               ap=[[1, 16], [16, JP // 16]]))
                widx.append(wi)
                nc.scalar.dma_start(out=sc_d[im, 0:J].rearrange("(c k) -> c k", k=K),
                                    in_=tv)
                nc.scalar.dma_start(out=sc_d[im, J:JP], in_=fpad_c[0])
                ssb = pp.tile([128, JP // 128], F32, tag=f"ssb_{im}")
                nc.scalar.dma_start(
                    out=ssb,
                    in_=AP(tensor=sc_d[:].tensor, offset=im * JP,
                           ap=[[1, 128], [128, JP // 128]]))
                score_sb.append(ssb)
                if im == 0:
                    mean128 = emit_ctr()

            # ---------------- feat gather / output ------------------------
            for im in range(IPC):
                pts = []
                for q in range(6):
                    pts.append(qp.tile([128, 256], F32, tag=f"pt{q}",
                                       name=f"pt{im}_{q}"))
                for t in range(2):
                    ft = fts[(im, t)]
                    if im > 0:
                        nc.sync.dma_start(out=ft,
                                          in_=feat_in[im, 128 * t:128 * (t + 1), :])
                    g = gp.tile([128, JP], F32, tag="g", name=f"g{im}_{t}")
                    nc.gpsimd.indirect_copy(out=g, data=ft, idxs=widx[im],
                                            i_know_ap_gather_is_preferred=True)
                    for q in range(6):
                        nc.tensor.transpose(out=pts[q][:, 128 * t:128 * (t + 1)],
                                            in_=g[:, 128 * q:128 * (q + 1)],
                                            identity=ident)
                for q in range(6):
                    osb = op.tile([128, 256], F32, tag="osb", name=f"osb{im}_{q}")
                    nc.scalar.activation(out=osb, in_=pts[q],
                                         func=mybir.ActivationFunctionType.Identity,
                                         bias=mean128[:, :1],
                                         scale=score_sb[im][:, q:q + 1])
                    if q < 5:
                        nc.sync.dma_start(out=out_o[im, 128 * q:128 * (q + 1), :],
                                          in_=osb)
                    else:
                        nc.sync.dma_start(out=out_o[im, 640:J, :],
                                          in_=osb[0:J - 640, :])
    nc.finalize()
    return nc


def kernel(heat: np.ndarray, feat: np.ndarray, reg_targets: np.ndarray) -> np.ndarray:
    heat = np.ascontiguousarray(heat, dtype=np.float32)
    feat = np.ascontiguousarray(feat, dtype=np.float32).reshape(B, CF, HW)
    regs = np.ascontiguousarray(reg_targets, dtype=np.float32)

    nc = build_program()
    in_maps = [
        {"heat": heat[c * IPC:(c + 1) * IPC],
         "feat": feat[c * IPC:(c + 1) * IPC],
         "regs": regs}
        for c in range(NCORES)
    ]
    res = run_bass_kernel_spmd(nc, in_maps, list(range(NCORES)))
    out = np.concatenate([np.asarray(r["out"]) for r in res.results], axis=0)
    return out.reshape(B, J, CF)


if __name__ == "__main__":
    import ref_numpy as RN
    inputs = RN.get_inputs()
    exp = RN.get_expected(inputs)
    got = kernel(inputs["heat"], inputs["feat"], inputs["reg_targets"])
    err = np.abs(got - exp).max() / np.abs(exp).max()
    print("Relative error:", err)


# revision 23
# speedup vs baseline: 1.2227x; 1.0765x over previous
"""Trainium2 Bass kernel for nn_DirectPoseOutputs (loss_fn).

Reference computation:
  1) 3x3 max-pool NMS on heat [16,17,100,152]
  2) per-channel top-40 (scores + flat indices), jax.lax.top_k tie order
  3) gather feat [16,256,15200] columns at the top-40 indices -> [B,680,256]
  4) ctrness mean over reg_targets [20000,4] (scalar)
  out = gathered * scores + mean_ctrness

Sharding: data-parallel over batch, 2 images per core, 8 cores. ctrness is
computed redundantly on every core (320KB read, no collectives).

Per-core pipeline (engine assignment in brackets):
  feat stream [SP queue only]: 4x [128,15200] tiles, ~21us each.
  heat path: DRAM->DRAM transpose to y-major [ACT], load ht + y-shifted
  copies [ACT]; separable NMS: H-max from shifted loads + W-max via free-dim
  shifts [DVE], mask (is_ge, mult) on [GPSIMD]; stage-1 per-y-row top-8
  via InstMax/InstMaxIndex [DVE]; stage-2 relayout to per-channel rows via
  PE transpose + coarse-grained fold DMA (m = s*100+y order; validated safe
  for the duplicate-value tie cases); stage-3 top-40 via 5 rounds of
  max/max_index/match_replace [DVE]; stage-4 global indices via 6 indirect
  DMAs of [128,1] (one offset per dest partition row -- HW semantics);
  feat gather via GPSIMD indirect_copy (shared wrapped index table);
  PE-transpose [c,j]->[j,c]; fused out = psum*score[j] + mean [ACT]; out
  DMAs [SP].
"""
import numpy as np

import concourse.bass as bass
import concourse.bacc as bacc
import concourse.mybir as mybir
from concourse.bass_types import AP
from concourse.bass_utils import run_bass_kernel_spmd
from concourse.masks import make_identity
from concourse.tile import TileContext

F32 = mybir.dt.float32
U16 = mybir.dt.uint16
U32 = mybir.dt.uint32

B, CK, H, W = 16, 17, 100, 152
HW = H * W            # 15200
CW = CK * W           # 2584 free width of the y-major heat tile
CF = 256
K = 40
NCORES = 8
IPC = B // NCORES     # images per core = 2
J = CK * K            # 680 output rows per image
JP = 768              # padded to 6 tiles of 128 rows (and %16 for the table)
NREG = 20000
MAX, GE, MUL, ADD = (mybir.AluOpType.max, mybir.AluOpType.is_ge,
                     mybir.AluOpType.mult, mybir.AluOpType.add)


def build_program() -> bass.Bass:
    nc = bacc.Bacc()

    heat_in = nc.declare_dram_parameter("heat", [IPC, CK, H, W], F32, isOutput=False)
    feat_in = nc.declare_dram_parameter("feat", [IPC, CF, HW], F32, isOutput=False)
    regs_in = nc.declare_dram_parameter("regs", [NREG, 4], F32, isOutput=False)
    out_o = nc.declare_dram_parameter("out", [IPC, J, CF], F32, isOutput=True)

    # constants
    rowbase_c = nc.inline_tensor(
        np.broadcast_to((np.arange(H, dtype=np.float32) * W)[:, None],
                        (H, CK * 8)).copy(), "rowbasec")
    chbase_c = nc.inline_tensor(
        np.broadcast_to((np.arange(CK, dtype=np.uint32) * 800)[:, None],
                        (CK, K)).copy(), "chbasec")
    zpad32_c = nc.inline_tensor(np.zeros((1, JP - J), np.uint32), "zpad32c")
    fpad_c = nc.inline_tensor(np.zeros((1, JP - J), np.float32), "fpadc")

    # DRAM scratch
    cd_d = [nc.dram_tensor(f"cd_d{i}", [136, H], F32) for i in range(IPC)]
    gd_d = [nc.dram_tensor(f"gd_d{i}", [136, H], F32) for i in range(IPC)]
    g16_d = [nc.dram_tensor(f"g16_d{i}", [CK * 800], U16) for i in range(IPC)]
    sc_d = nc.dram_tensor("sc_d", [IPC, JP], F32)            # scores bounce
    si_d = nc.dram_tensor("si_d", [IPC, JP], U16)            # gather table bounce
    so_d = nc.dram_tensor("so_d", [IPC, JP], U32)            # stage-4 offsets

    with TileContext(nc) as tc:
        with (
            tc.tile_pool(name="consts", bufs=1) as cp,
            tc.tile_pool(name="pers", bufs=1) as pp,
            tc.tile_pool(name="heat", bufs=1) as hp,
            tc.tile_pool(name="ft", bufs=2) as fp,
            tc.tile_pool(name="gt", bufs=2) as gp,
            tc.tile_pool(name="osb", bufs=4) as op,
            tc.tile_pool(name="ps", bufs=1, space="PSUM") as qp,
            tc.tile_pool(name="psc", bufs=1, space="PSUM") as qcp,
        ):
            # ---- feat stream + heat prefetches start immediately ---------
            fts = {}

            def feat_load(im, t):
                ft = fts[(im, t)]
                for h in range(4):
                    nc.sync.dma_start(
                        out=ft[:, 3800 * h:3800 * (h + 1)],
                        in_=feat_in[im, 128 * t:128 * (t + 1),
                                    3800 * h:3800 * (h + 1)])

            for im in range(IPC):
                for t in range(2):
                    fts[(im, t)] = fp.tile([128, HW], F32, tag="ft",
                                           name=f"ft{im}_{t}")
            for t in range(2):
                feat_load(0, t)
            def emit_ctr():
                rt = cp.tile([125, 160, 4], F32, tag="regs", name="rt")
                nc.sync.dma_start(out=rt,
                                  in_=regs_in[:].rearrange("(p a) b -> p a b", p=125))
                l_, t_, r_, b_ = (rt[:, :, i] for i in range(4))
                mn = cp.tile([125, 160], F32, tag="ctr_mn", name="ctr_mn")
                mx = cp.tile([125, 160], F32, tag="ctr_mx", name="ctr_mx")
                pr = cp.tile([125, 160], F32, tag="ctr_pr", name="ctr_pr")
                nc.vector.tensor_tensor(out=mn, in0=l_, in1=r_, op=mybir.AluOpType.min)
                nc.vector.tensor_tensor(out=mx, in0=l_, in1=r_, op=MAX)
                nc.vector.reciprocal(out=mx, in_=mx)
                nc.vector.tensor_tensor(out=pr, in0=mn, in1=mx, op=MUL)
                nc.vector.tensor_tensor(out=mn, in0=t_, in1=b_, op=mybir.AluOpType.min)
                nc.vector.tensor_tensor(out=mx, in0=t_, in1=b_, op=MAX)
                nc.vector.reciprocal(out=mx, in_=mx)
                nc.vector.tensor_tensor(out=mn, in0=mn, in1=mx, op=MUL)
                nc.vector.tensor_tensor(out=pr, in0=pr, in1=mn, op=MUL)
                acc = cp.tile([128, 1], F32, tag="ctr_acc", name="ctr_acc")
                nc.vector.memset(acc, 0.0)
                ctr_s = cp.tile([125, 160], F32, tag="ctr_s", name="ctr_s")
                nc.scalar.activation(out=ctr_s, in_=pr,
                                     func=mybir.ActivationFunctionType.Sqrt,
                                     accum_out=acc[0:125, :])
                ones_col = cp.tile([128, 1], F32, tag="ones_col", name="ones_col")
                nc.vector.memset(ones_col, 1.0)
                ones_row = cp.tile([1, 128], F32, tag="ones_row", name="ones_row")
                nc.vector.memset(ones_row, 1.0)
                tot_p = qcp.tile([1, 1], F32, tag="aux0", name="tot_p")
                nc.tensor.matmul(tot_p, ones_col, acc, start=True, stop=True)
                mean1 = cp.tile([1, 1], F32, tag="mean1", name="mean1")
                nc.scalar.activation(out=mean1, in_=tot_p,
                                     func=mybir.ActivationFunctionType.Copy,
                                     scale=1.0 / NREG)
                mean_p = qcp.tile([128, 1], F32, tag="aux1", name="mean_p")
                nc.tensor.matmul(mean_p, ones_row, mean1, start=True, stop=True)
                mean128 = cp.tile([128, 1], F32, tag="mean128", name="mean128")
                nc.vector.tensor_copy(out=mean128, in_=mean_p)
                return mean128

            ident = cp.tile([128, 128], F32, tag="ident")
            make_identity(nc, ident)
            rowbase = cp.tile([H, CK, 8], F32, tag="rowbase")
            nc.sync.dma_start(out=rowbase, in_=rowbase_c[:].rearrange(
                "h (c s) -> h c s", s=8))
            chbase = cp.tile([CK, K], U32, tag="chbase")
            nc.sync.dma_start(out=chbase, in_=chbase_c[:])
            mean128 = None

            score_sb = []
            widx = []
            all_pts = []
            # ---------------- heat pipeline, one image at a time ----------
            for im in range(IPC):
                ldq = nc.scalar if im == 0 else nc.sync
                shq = nc.gpsimd if im == 0 else nc.sync
                ht = hp.tile([H, CK, W], F32, tag="ht")
                hu = hp.tile([H, CK, W], F32, tag="hu")
                hd = hp.tile([H, CK, W], F32, tag="hd")
                for ch in range(CK):
                    ldq.dma_start(out=ht[:, ch, :], in_=heat_in[im, ch])
                # y-shifted copies with clamped edges (SBUF->SBUF)
                shq.dma_start(out=hu[0:H - 1], in_=ht[1:H])
                shq.dma_start(out=hu[H - 1:H], in_=ht[H - 1:H])
                shq.dma_start(out=hd[1:H], in_=ht[0:H - 1])
                shq.dma_start(out=hd[0:1], in_=ht[0:1])
                # H-direction 3-max into hu
                nc.vector.tensor_tensor(out=hu, in0=hu, in1=ht, op=MAX)
                nc.vector.tensor_tensor(out=hu, in0=hu, in1=hd, op=MAX)
                # W-direction 3-max into hd
                nc.vector.tensor_tensor(out=hd[:, :, 0:W - 1], in0=hu[:, :, 0:W - 1],
                                        in1=hu[:, :, 1:W], op=MAX)
                nc.vector.tensor_copy(out=hd[:, :, W - 1:W], in_=hu[:, :, W - 1:W])
                nc.vector.tensor_tensor(out=hd[:, :, 1:W], in0=hd[:, :, 1:W],
                                        in1=hu[:, :, 0:W - 1], op=MAX)
                # keep only local maxima: ht *= (ht >= hd)
                nc.vector.tensor_tensor(out=hu, in0=ht, in1=hd, op=GE)
                nc.gpsimd.tensor_tensor(out=ht, in0=ht, in1=hu, op=MUL)

                # stage 1: per-y-row top-8 values + row-local indices
                vals8 = pp.tile([H, CK, 8], F32, tag=f"vals8_{im}")
                rl8 = pp.tile([H, CK, 8], U16, tag=f"rl8_{im}")
                for ch in range(CK):
                    nc.vector.max(out=vals8[:, ch, :], in_=ht[:, ch, :])
                    nc.vector.max_index(out=rl8[:, ch, :], in_max=vals8[:, ch, :],
                                        in_values=ht[:, ch, :])
                # global flat index, in f32 (exact below 2^24)
                gidxf = pp.tile([H, CK, 8], F32, tag=f"gidxf_{im}")
                nc.vector.tensor_copy(out=gidxf, in_=rl8)
                nc.vector.tensor_tensor(out=gidxf, in0=gidxf, in1=rowbase, op=ADD)

                # stage 2: transpose candidates to per-channel rows via PE.
                # m-order: m = s*100 + y  (cand[ch, m] = vals8[y, ch, s])
                cand = pp.tile([CK, 800], F32, tag=f"cand_{im}")
                g16f = pp.tile([CK, 800], F32, tag=f"g16f_{im}")
                for srct, dst, dscr in ((vals8, cand, cd_d[im]),
                                        (gidxf, g16f, gd_d[im])):
                    for blk in range(2):
                        p0, pn = blk * 128, min(136 - blk * 128, 128)
                        tp = qcp.tile([128, H], F32, tag=f"aux{blk}",
                                      name=f"st2_{im}_{blk}_{dst.tensor.name}")
                        nc.tensor.transpose(
                            out=tp[0:pn, :],
                            in_=srct[:].rearrange("h c s -> h (c s)")[:, p0:p0 + pn],
                            identity=ident[0:H, 0:H])
                        tps = pp.tile([128, H], F32, tag=f"st2s_{blk}",
                                      name=f"st2s_{im}_{blk}_{dst.tensor.name}")
                        nc.vector.tensor_copy(out=tps[0:pn, :], in_=tp[0:pn, :])
                        nc.scalar.dma_start(out=dscr[p0:p0 + pn, :], in_=tps[0:pn, :])
                    # dst[ch, s*100+y] = dscr[ch*8+s, y]
                    nc.scalar.dma_start(
                        out=dst,
                        in_=dscr[:].rearrange("(c s) h -> c (s h)", s=8))

                # stage 3: per-channel top-40 of the 800 candidates
                tv = pp.tile([CK, K], F32, tag=f"tv_{im}")
                cpos = pp.tile([CK, K], U32, tag=f"cpos_{im}")
                for r in range(5):
                    v8 = tv[:, r * 8:(r + 1) * 8]
                    nc.vector.max(out=v8, in_=cand)
                    nc.vector.max_index(out=cpos[:, r * 8:(r + 1) * 8], in_max=v8,
                                        in_values=cand)
                    nc.vector.match_replace(out=cand, in_to_replace=v8,
                                            in_values=cand, imm_value=-1.0)

                # stage 4: global indices.  g16f (f32) -> u16 -> DRAM
                # (contiguous), then 6 indirect row-gathers of [128, 1].
                g16u = pp.tile([CK, 800], U16, tag=f"g16u_{im}")
                nc.vector.tensor_copy(out=g16u, in_=g16f)
                nc.scalar.dma_start(out=g16_d[im][:].rearrange("(c m) -> c m", m=800),
                                    in_=g16u)
                offs = pp.tile([CK, K], U32, tag=f"offs_{im}")
                nc.vector.tensor_tensor(out=offs, in0=cpos, in1=chbase, op=ADD)
                nc.scalar.dma_start(out=so_d[im, 0:J].rearrange("(c k) -> c k", k=K),
                                    in_=offs)
                nc.scalar.dma_start(out=so_d[im, J:JP], in_=zpad32_c[0])
                o128 = pp.tile([128, JP // 128], U32, tag=f"o128_{im}")
                nc.scalar.dma_start(
                    out=o128,
                    in_=AP(tensor=so_d[:].tensor, offset=im * JP,
                           ap=[[1, 128], [128, JP // 128]]))
                gi128 = pp.tile([128, JP // 128], U16, tag=f"gi128_{im}")
                for q in range(JP // 128):
                    nc.gpsimd.indirect_dma_start(
                        out=gi128[:, q:q + 1], out_offset=None,
                        in_=AP(tensor=g16_d[im][:].tensor, offset=0,
                               ap=[[1, CK * 800], [1, 1]]),
                        in_offset=bass.IndirectOffsetOnAxis(ap=o128[:, q:q + 1],
                                                            axis=0))

                # stage 5: wrapped gather table + per-tile scores
                nc.scalar.dma_start(
                    out=AP(tensor=si_d[:].tensor, offset=im * JP,
                           ap=[[1, 128], [128, JP // 128]]),
                    in_=gi128)
                wi = pp.tile([128, JP // 16], U16, tag=f"widx_{im}")
                for grp in range(8):
                    nc.scalar.dma_start(
                        out=wi[16 * grp:16 * (grp + 1), :],
                        in_=AP(tensor=si_d[:].tensor, offset=im * JP,
                               ap=[[1, 16], [16, JP // 16]]))
                widx.append(wi)
                nc.scalar.dma_start(out=sc_d[im, 0:J].rearrange("(c k) -> c k", k=K),
                                    in_=tv)
                nc.scalar.dma_start(out=sc_d[im, J:JP], in_=fpad_c[0])
                ssb = pp.tile([128, JP // 128], F32, tag=f"ssb_{im}")
                nc.scalar.dma_start(
                    out=ssb,
                    in_=AP(tensor=sc_d[:].tensor, offset=im * JP,
                           ap=[[1, 128], [128, JP // 128]]))
                score_sb.append(ssb)
                if im == 0:
                    mean128 = emit_ctr()

                # ---- gather + transpose for this image ----
                pts = []
                for q in range(6):
                    pts.append(qp.tile([128, 256], F32, tag=f"pt{q}",
                                       name=f"pt{im}_{q}"))
                all_pts.append(pts)
                for t in range(2):
                    ft = fts[(im, t)]
                    if im > 0:
                        feat_load(im, t)
                    g = gp.tile([128, JP], F32, tag="g", name=f"g{im}_{t}")
                    nc.gpsimd.indirect_copy(out=g, data=ft, idxs=wi,
                                            i_know_ap_gather_is_preferred=True)
                    for q in range(6):
                        nc.tensor.transpose(out=pts[q][:, 128 * t:128 * (t + 1)],
                                            in_=g[:, 128 * q:128 * (q + 1)],
                                            identity=ident)

            # ---------------- scale/bias + output -------------------------
            for im in range(IPC):
                for q in range(6):
                    osb = op.tile([128, 256], F32, tag="osb", name=f"osb{im}_{q}")
                    nc.scalar.activation(out=osb, in_=all_pts[im][q],
                                         func=mybir.ActivationFunctionType.Identity,
                                         bias=mean128[:, :1],
                                         scale=score_sb[im][:, q:q + 1])
                    if q < 5:
                        nc.sync.dma_start(out=out_o[im, 128 * q:128 * (q + 1), :],
                                          in_=osb)
                    else:
                        nc.sync.dma_start(out=out_o[im, 640:J, :],
                                          in_=osb[0:J - 640, :])
    nc.finalize()
    return nc


def kernel(heat: np.ndarray, feat: np.ndarray, reg_targets: np.ndarray) -> np.ndarray:
    heat = np.ascontiguousarray(heat, dtype=np.float32)
    feat = np.ascontiguousarray(feat, dtype=np.float32).reshape(B, CF, HW)
    regs = np.ascontiguousarray(reg_targets, dtype=np.float32)

    nc = build_program()
    in_maps = [
        {"heat": heat[c * IPC:(c + 1) * IPC],
         "feat": feat[c * IPC:(c + 1) * IPC],
         "regs": regs}
        for c in range(NCORES)
    ]
    res = run_bass_kernel_spmd(nc, in_maps, list(range(NCORES)))
    out = np.concatenate([np.asarray(r["out"]) for r in res.results], axis=0)
    return out.reshape(B, J, CF)


if __name__ == "__main__":
    import ref_numpy as RN
    inputs = RN.get_inputs()
    exp = RN.get_expected(inputs)
    got = kernel(inputs["heat"], inputs["feat"], inputs["reg_targets"])
    err = np.abs(got - exp).max() / np.abs(exp).max()
    print("Relative error:", err)


# revision 24
# speedup vs baseline: 1.3642x; 1.1158x over previous
"""Trainium2 Bass kernel for nn_DirectPoseOutputs (loss_fn).

Reference computation:
  1) 3x3 max-pool NMS on heat [16,17,100,152]
  2) per-channel top-40 (scores + flat indices), jax.lax.top_k tie order
  3) gather feat [16,256,15200] columns at the top-40 indices -> [B,680,256]
  4) ctrness mean over reg_targets [20000,4] (scalar)
  out = gathered * scores + mean_ctrness

Sharding: data-parallel over batch, 2 images per core, 8 cores. ctrness is
computed redundantly on every core (320KB read, no collectives).

Per-core pipeline (engine assignment in brackets):
  feat stream [SP queue only]: 4x [128,15200] tiles, ~21us each.
  heat path: DRAM->DRAM transpose to y-major [ACT], load ht + y-shifted
  copies [ACT]; separable NMS: H-max from shifted loads + W-max via free-dim
  shifts [DVE], mask (is_ge, mult) on [GPSIMD]; stage-1 per-y-row top-8
  via InstMax/InstMaxIndex [DVE]; stage-2 relayout to per-channel rows via
  PE transpose + coarse-grained fold DMA (m = s*100+y order; validated safe
  for the duplicate-value tie cases); stage-3 top-40 via 5 rounds of
  max/max_index/match_replace [DVE]; stage-4 global indices via 6 indirect
  DMAs of [128,1] (one offset per dest partition row -- HW semantics);
  feat gather via GPSIMD indirect_copy (shared wrapped index table);
  PE-transpose [c,j]->[j,c]; fused out = psum*score[j] + mean [ACT]; out
  DMAs [SP].
"""
import numpy as np

import concourse.bass as bass
import concourse.bacc as bacc
import concourse.mybir as mybir
from concourse.bass_types import AP
from concourse.bass_utils import run_bass_kernel_spmd
from concourse.masks import make_identity
from concourse.tile import TileContext, add_dep_helper

F32 = mybir.dt.float32
U16 = mybir.dt.uint16
U32 = mybir.dt.uint32

B, CK, H, W = 16, 17, 100, 152
HW = H * W            # 15200
CW = CK * W           # 2584 free width of the y-major heat tile
CF = 256
K = 40
NCORES = 8
IPC = B // NCORES     # images per core = 2
J = CK * K            # 680 output rows per image
JP = 768              # padded to 6 tiles of 128 rows (and %16 for the table)
NREG = 20000
MAX, GE, MUL, ADD = (mybir.AluOpType.max, mybir.AluOpType.is_ge,
                     mybir.AluOpType.mult, mybir.AluOpType.add)


def build_program() -> bass.Bass:
    nc = bacc.Bacc()

    heat_in = nc.declare_dram_parameter("heat", [IPC, CK, H, W], F32, isOutput=False)
    feat_in = nc.declare_dram_parameter("feat", [IPC, CF, HW], F32, isOutput=False)
    regs_in = nc.declare_dram_parameter("regs", [NREG, 4], F32, isOutput=False)
    out_o = nc.declare_dram_parameter("out", [IPC, J, CF], F32, isOutput=True)

    # constants
    rowbase_c = nc.inline_tensor(
        np.broadcast_to((np.arange(H, dtype=np.float32) * W)[:, None],
                        (H, CK * 8)).copy(), "rowbasec")
    chbase_c = nc.inline_tensor(
        np.broadcast_to((np.arange(CK, dtype=np.uint32) * 800)[:, None],
                        (CK, K)).copy(), "chbasec")
    zpad32_c = nc.inline_tensor(np.zeros((1, JP - J), np.uint32), "zpad32c")
    fpad_c = nc.inline_tensor(np.zeros((1, JP - J), np.float32), "fpadc")

    # DRAM scratch
    cd_d = [nc.dram_tensor(f"cd_d{i}", [136, H], F32) for i in range(IPC)]
    gd_d = [nc.dram_tensor(f"gd_d{i}", [136, H], F32) for i in range(IPC)]
    g16_d = [nc.dram_tensor(f"g16_d{i}", [CK * 800], U16) for i in range(IPC)]
    sc_d = nc.dram_tensor("sc_d", [IPC, JP], F32)            # scores bounce
    si_d = nc.dram_tensor("si_d", [IPC, JP], U16)            # gather table bounce
    so_d = nc.dram_tensor("so_d", [IPC, JP], U32)            # stage-4 offsets

    with TileContext(nc) as tc:
        with (
            tc.tile_pool(name="consts", bufs=1) as cp,
            tc.tile_pool(name="pers", bufs=1) as pp,
            tc.tile_pool(name="heat", bufs=1) as hp,
            tc.tile_pool(name="ft", bufs=2) as fp,
            tc.tile_pool(name="gt", bufs=2) as gp,
            tc.tile_pool(name="osb", bufs=4) as op,
            tc.tile_pool(name="ps", bufs=1, space="PSUM") as qp,
            tc.tile_pool(name="psc", bufs=1, space="PSUM") as qcp,
        ):
            # ---- feat stream + heat prefetches start immediately ---------
            fts = {}
            heat_tiles = {}
            heat_last = []

            # heat loads first: their small descriptors must not queue behind
            # the bulk feat descriptors on the DMA engines.
            for im in range(IPC):
                ldq = nc.scalar if im == 0 else nc.sync
                ht = hp.tile([H, CK, W], F32, tag="ht", bufs=2, name=f"ht{im}")
                heat_tiles[im] = ht
                last = None
                for ch in range(CK):
                    last = ldq.dma_start(out=ht[:, ch, :], in_=heat_in[im, ch])
                heat_last.append(last.ins)

            def feat_load(im, t):
                ft = fts[(im, t)]
                for h in range(4):
                    d = nc.sync.dma_start(
                        out=ft[:, 3800 * h:3800 * (h + 1)],
                        in_=feat_in[im, 128 * t:128 * (t + 1),
                                    3800 * h:3800 * (h + 1)])
                    for hl in heat_last:
                        add_dep_helper(d.ins, hl, sync=True,
                                       reason="feat waits for heat stream")

            for im in range(IPC):
                for t in range(2):
                    fts[(im, t)] = fp.tile([128, HW], F32, tag="ft",
                                           name=f"ft{im}_{t}")
            for t in range(2):
                feat_load(0, t)
            def emit_ctr():
                rt = cp.tile([125, 160, 4], F32, tag="regs", name="rt")
                nc.sync.dma_start(out=rt,
                                  in_=regs_in[:].rearrange("(p a) b -> p a b", p=125))
                l_, t_, r_, b_ = (rt[:, :, i] for i in range(4))
                mn = cp.tile([125, 160], F32, tag="ctr_mn", name="ctr_mn")
                mx = cp.tile([125, 160], F32, tag="ctr_mx", name="ctr_mx")
                pr = cp.tile([125, 160], F32, tag="ctr_pr", name="ctr_pr")
                nc.vector.tensor_tensor(out=mn, in0=l_, in1=r_, op=mybir.AluOpType.min)
                nc.vector.tensor_tensor(out=mx, in0=l_, in1=r_, op=MAX)
                nc.vector.reciprocal(out=mx, in_=mx)
                nc.vector.tensor_tensor(out=pr, in0=mn, in1=mx, op=MUL)
                nc.vector.tensor_tensor(out=mn, in0=t_, in1=b_, op=mybir.AluOpType.min)
                nc.vector.tensor_tensor(out=mx, in0=t_, in1=b_, op=MAX)
                nc.vector.reciprocal(out=mx, in_=mx)
                nc.vector.tensor_tensor(out=mn, in0=mn, in1=mx, op=MUL)
                nc.vector.tensor_tensor(out=pr, in0=pr, in1=mn, op=MUL)
                acc = cp.tile([128, 1], F32, tag="ctr_acc", name="ctr_acc")
                nc.vector.memset(acc, 0.0)
                ctr_s = cp.tile([125, 160], F32, tag="ctr_s", name="ctr_s")
                nc.scalar.activation(out=ctr_s, in_=pr,
                                     func=mybir.ActivationFunctionType.Sqrt,
                                     accum_out=acc[0:125, :])
                ones_col = cp.tile([128, 1], F32, tag="ones_col", name="ones_col")
                nc.vector.memset(ones_col, 1.0)
                ones_row = cp.tile([1, 128], F32, tag="ones_row", name="ones_row")
                nc.vector.memset(ones_row, 1.0)
                tot_p = qcp.tile([1, 1], F32, tag="aux0", name="tot_p")
                nc.tensor.matmul(tot_p, ones_col, acc, start=True, stop=True)
                mean1 = cp.tile([1, 1], F32, tag="mean1", name="mean1")
                nc.scalar.activation(out=mean1, in_=tot_p,
                                     func=mybir.ActivationFunctionType.Copy,
                                     scale=1.0 / NREG)
                mean_p = qcp.tile([128, 1], F32, tag="aux1", name="mean_p")
                nc.tensor.matmul(mean_p, ones_row, mean1, start=True, stop=True)
                mean128 = cp.tile([128, 1], F32, tag="mean128", name="mean128")
                nc.vector.tensor_copy(out=mean128, in_=mean_p)
                return mean128

            ident = cp.tile([128, 128], F32, tag="ident")
            make_identity(nc, ident)
            rowbase = cp.tile([H, CK, 8], F32, tag="rowbase")
            nc.sync.dma_start(out=rowbase, in_=rowbase_c[:].rearrange(
                "h (c s) -> h c s", s=8))
            chbase = cp.tile([CK, K], U32, tag="chbase")
            nc.sync.dma_start(out=chbase, in_=chbase_c[:])
            mean128 = None

            score_sb = []
            widx = []
            all_pts = []
            # ---------------- heat pipeline, one image at a time ----------
            for im in range(IPC):
                shq = nc.gpsimd if im == 0 else nc.scalar
                ht = heat_tiles[im]
                hu = hp.tile([H, CK, W], F32, tag="hu")
                hd = hp.tile([H, CK, W], F32, tag="hd")
                # y-shifted copies with clamped edges (SBUF->SBUF)
                shq.dma_start(out=hu[0:H - 1], in_=ht[1:H])
                shq.dma_start(out=hu[H - 1:H], in_=ht[H - 1:H])
                shq.dma_start(out=hd[1:H], in_=ht[0:H - 1])
                shq.dma_start(out=hd[0:1], in_=ht[0:1])
                # H-direction 3-max into hu
                nc.vector.tensor_tensor(out=hu, in0=hu, in1=ht, op=MAX)
                nc.vector.tensor_tensor(out=hu, in0=hu, in1=hd, op=MAX)
                # W-direction 3-max into hd
                nc.vector.tensor_tensor(out=hd[:, :, 0:W - 1], in0=hu[:, :, 0:W - 1],
                                        in1=hu[:, :, 1:W], op=MAX)
                nc.vector.tensor_copy(out=hd[:, :, W - 1:W], in_=hu[:, :, W - 1:W])
                nc.vector.tensor_tensor(out=hd[:, :, 1:W], in0=hd[:, :, 1:W],
                                        in1=hu[:, :, 0:W - 1], op=MAX)
                # keep only local maxima: ht *= (ht >= hd)
                nc.vector.tensor_tensor(out=hu, in0=ht, in1=hd, op=GE)
                nc.gpsimd.tensor_tensor(out=ht, in0=ht, in1=hu, op=MUL)

                # stage 1: per-y-row top-8 values + row-local indices
                vals8 = pp.tile([H, CK, 8], F32, tag="vals8", name=f"vals8_{im}")
                rl8 = pp.tile([H, CK, 8], U16, tag="rl8", name=f"rl8_{im}")
                for ch in range(CK):
                    nc.vector.max(out=vals8[:, ch, :], in_=ht[:, ch, :])
                    nc.vector.max_index(out=rl8[:, ch, :], in_max=vals8[:, ch, :],
                                        in_values=ht[:, ch, :])
                # global flat index, in f32 (exact below 2^24)
                gidxf = pp.tile([H, CK, 8], F32, tag="gidxf", name=f"gidxf_{im}")
                nc.vector.tensor_copy(out=gidxf, in_=rl8)
                nc.vector.tensor_tensor(out=gidxf, in0=gidxf, in1=rowbase, op=ADD)

                # stage 2: transpose candidates to per-channel rows via PE.
                # m-order: m = s*100 + y  (cand[ch, m] = vals8[y, ch, s])
                cand = pp.tile([CK, 800], F32, tag="cand", name=f"cand_{im}")
                g16f = pp.tile([CK, 800], F32, tag="g16f", name=f"g16f_{im}")
                for srct, dst, dscr in ((vals8, cand, cd_d[im]),
                                        (gidxf, g16f, gd_d[im])):
                    for blk in range(2):
                        p0, pn = blk * 128, min(136 - blk * 128, 128)
                        tp = qcp.tile([128, H], F32, tag=f"aux{blk}",
                                      name=f"st2_{im}_{blk}_{dst.tensor.name}")
                        nc.tensor.transpose(
                            out=tp[0:pn, :],
                            in_=srct[:].rearrange("h c s -> h (c s)")[:, p0:p0 + pn],
                            identity=ident[0:H, 0:H])
                        tps = pp.tile([128, H], F32, tag=f"st2s_{blk}",
                                      name=f"st2s_{im}_{blk}_{dst.tensor.name}")
                        nc.vector.tensor_copy(out=tps[0:pn, :], in_=tp[0:pn, :])
                        nc.scalar.dma_start(out=dscr[p0:p0 + pn, :], in_=tps[0:pn, :])
                    # dst[ch, s*100+y] = dscr[ch*8+s, y]
                    nc.scalar.dma_start(
                        out=dst,
                        in_=dscr[:].rearrange("(c s) h -> c (s h)", s=8))

                # stage 3: per-channel top-40 of the 800 candidates
                tv = pp.tile([CK, K], F32, tag=f"tv_{im}")
                cpos = pp.tile([CK, K], U32, tag=f"cpos_{im}")
                for r in range(5):
                    v8 = tv[:, r * 8:(r + 1) * 8]
                    nc.vector.max(out=v8, in_=cand)
                    nc.vector.max_index(out=cpos[:, r * 8:(r + 1) * 8], in_max=v8,
                                        in_values=cand)
                    nc.vector.match_replace(out=cand, in_to_replace=v8,
                                            in_values=cand, imm_value=-1.0)

                # stage 4: global indices.  g16f (f32) -> u16 -> DRAM
                # (contiguous), then 6 indirect row-gathers of [128, 1].
                g16u = pp.tile([CK, 800], U16, tag="g16u", name=f"g16u_{im}")
                nc.vector.tensor_copy(out=g16u, in_=g16f)
                nc.scalar.dma_start(out=g16_d[im][:].rearrange("(c m) -> c m", m=800),
                                    in_=g16u)
                offs = pp.tile([CK, K], U32, tag=f"offs_{im}")
                nc.vector.tensor_tensor(out=offs, in0=cpos, in1=chbase, op=ADD)
                nc.scalar.dma_start(out=so_d[im, 0:J].rearrange("(c k) -> c k", k=K),
                                    in_=offs)
                nc.scalar.dma_start(out=so_d[im, J:JP], in_=zpad32_c[0])
                o128 = pp.tile([128, JP // 128], U32, tag=f"o128_{im}")
                nc.scalar.dma_start(
                    out=o128,
                    in_=AP(tensor=so_d[:].tensor, offset=im * JP,
                           ap=[[1, 128], [128, JP // 128]]))
                gi128 = pp.tile([128, JP // 128], U16, tag=f"gi128_{im}")
                for q in range(JP // 128):
                    nc.gpsimd.indirect_dma_start(
                        out=gi128[:, q:q + 1], out_offset=None,
                        in_=AP(tensor=g16_d[im][:].tensor, offset=0,
                               ap=[[1, CK * 800], [1, 1]]),
                        in_offset=bass.IndirectOffsetOnAxis(ap=o128[:, q:q + 1],
                                                            axis=0))

                # stage 5: wrapped gather table + per-tile scores
                nc.scalar.dma_start(
                    out=AP(tensor=si_d[:].tensor, offset=im * JP,
                           ap=[[1, 128], [128, JP // 128]]),
                    in_=gi128)
                wi = pp.tile([128, JP // 16], U16, tag=f"widx_{im}")
                for grp in range(8):
                    nc.scalar.dma_start(
                        out=wi[16 * grp:16 * (grp + 1), :],
                        in_=AP(tensor=si_d[:].tensor, offset=im * JP,
                               ap=[[1, 16], [16, JP // 16]]))
                widx.append(wi)
                nc.scalar.dma_start(out=sc_d[im, 0:J].rearrange("(c k) -> c k", k=K),
                                    in_=tv)
                nc.scalar.dma_start(out=sc_d[im, J:JP], in_=fpad_c[0])
                ssb = pp.tile([128, JP // 128], F32, tag=f"ssb_{im}")
                nc.scalar.dma_start(
                    out=ssb,
                    in_=AP(tensor=sc_d[:].tensor, offset=im * JP,
                           ap=[[1, 128], [128, JP // 128]]))
                score_sb.append(ssb)
                if im == 0:
                    mean128 = emit_ctr()

                # ---- gather + transpose for this image ----
                pts = []
                for q in range(6):
                    pts.append(qp.tile([128, 256], F32, tag=f"pt{q}",
                                       name=f"pt{im}_{q}"))
                all_pts.append(pts)
                for t in range(2):
                    ft = fts[(im, t)]
                    if im > 0:
                        feat_load(im, t)
                    g = gp.tile([128, JP], F32, tag="g", name=f"g{im}_{t}")
                    nc.gpsimd.indirect_copy(out=g, data=ft, idxs=wi,
                                            i_know_ap_gather_is_preferred=True)
                    for q in range(6):
                        nc.tensor.transpose(out=pts[q][:, 128 * t:128 * (t + 1)],
                                            in_=g[:, 128 * q:128 * (q + 1)],
                                            identity=ident)

            # ---------------- scale/bias + output -------------------------
            for im in range(IPC):
                for q in range(6):
                    osb = op.tile([128, 256], F32, tag="osb", name=f"osb{im}_{q}")
                    nc.scalar.activation(out=osb, in_=all_pts[im][q],
                                         func=mybir.ActivationFunctionType.Identity,
                                         bias=mean128[:, :1],
                                         scale=score_sb[im][:, q:q + 1])
                    if q < 5:
                        nc.sync.dma_start(out=out_o[im, 128 * q:128 * (q + 1), :],
                                          in_=osb)
                    else:
                        nc.sync.dma_start(out=out_o[im, 640:J, :],
                                          in_=osb[0:J - 640, :])
    nc.finalize()
    return nc


def kernel(heat: np.ndarray, feat: np.ndarray, reg_targets: np.ndarray) -> np.ndarray:
    heat = np.ascontiguousarray(heat, dtype=np.float32)
    feat = np.ascontiguousarray(feat, dtype=np.float32).reshape(B, CF, HW)
    regs = np.ascontiguousarray(reg_targets, dtype=np.float32)

    nc = build_program()
    in_maps = [
        {"heat": heat[c * IPC:(c + 1) * IPC],
         "feat": feat[c * IPC:(c + 1) * IPC],
         "regs": regs}
        for c in range(NCORES)
    ]
    res = run_bass_kernel_spmd(nc, in_maps, list(range(NCORES)))
    out = np.concatenate([np.asarray(r["out"]) for r in res.results], axis=0)
    return out.reshape(B, J, CF)


if __name__ == "__main__":
    import ref_numpy as RN
    inputs = RN.get_inputs()
    exp = RN.get_expected(inputs)
    got = kernel(inputs["heat"], inputs["feat"], inputs["reg_targets"])
    err = np.abs(got - exp).max() / np.abs(exp).max()
    print("Relative error:", err)


# revision 26
# speedup vs baseline: 1.4057x; 1.0304x over previous
"""Trainium2 Bass kernel for nn_DirectPoseOutputs (loss_fn).

Reference computation:
  1) 3x3 max-pool NMS on heat [16,17,100,152]
  2) per-channel top-40 (scores + flat indices), jax.lax.top_k tie order
  3) gather feat [16,256,15200] columns at the top-40 indices -> [B,680,256]
  4) ctrness mean over reg_targets [20000,4] (scalar)
  out = gathered * scores + mean_ctrness

Sharding: data-parallel over batch, 2 images per core, 8 cores. ctrness is
computed redundantly on every core (320KB read, no collectives).

Per-core pipeline (engine assignment in brackets):
  feat stream [SP queue only]: 4x [128,15200] tiles, ~21us each.
  heat path: DRAM->DRAM transpose to y-major [ACT], load ht + y-shifted
  copies [ACT]; separable NMS: H-max from shifted loads + W-max via free-dim
  shifts [DVE], mask (is_ge, mult) on [GPSIMD]; stage-1 per-y-row top-8
  via InstMax/InstMaxIndex [DVE]; stage-2 relayout to per-channel rows via
  PE transpose + coarse-grained fold DMA (m = s*100+y order; validated safe
  for the duplicate-value tie cases); stage-3 top-40 via 5 rounds of
  max/max_index/match_replace [DVE]; stage-4 global indices via 6 indirect
  DMAs of [128,1] (one offset per dest partition row -- HW semantics);
  feat gather via GPSIMD indirect_copy (shared wrapped index table);
  PE-transpose [c,j]->[j,c]; fused out = psum*score[j] + mean [ACT]; out
  DMAs [SP].
"""
import numpy as np

import concourse.bass as bass
import concourse.bacc as bacc
import concourse.mybir as mybir
from concourse.bass_types import AP
from concourse.bass_utils import run_bass_kernel_spmd
from concourse.masks import make_identity
from concourse.tile import TileContext, add_dep_helper

F32 = mybir.dt.float32
U16 = mybir.dt.uint16
U32 = mybir.dt.uint32

B, CK, H, W = 16, 17, 100, 152
HW = H * W            # 15200
CW = CK * W           # 2584 free width of the y-major heat tile
CF = 256
K = 40
NCORES = 8
IPC = B // NCORES     # images per core = 2
J = CK * K            # 680 output rows per image
JP = 768              # padded to 6 tiles of 128 rows (and %16 for the table)
NREG = 20000
MAX, GE, MUL, ADD = (mybir.AluOpType.max, mybir.AluOpType.is_ge,
                     mybir.AluOpType.mult, mybir.AluOpType.add)


def build_program() -> bass.Bass:
    nc = bacc.Bacc()

    heat_in = nc.declare_dram_parameter("heat", [IPC, CK, H, W], F32, isOutput=False)
    feat_in = nc.declare_dram_parameter("feat", [IPC, CF, HW], F32, isOutput=False)
    regs_in = nc.declare_dram_parameter("regs", [NREG, 4], F32, isOutput=False)
    out_o = nc.declare_dram_parameter("out", [IPC, J, CF], F32, isOutput=True)

    # constants
    rowbase_c = nc.inline_tensor(
        np.broadcast_to((np.arange(H, dtype=np.float32) * W)[:, None],
                        (H, CK * 8)).copy(), "rowbasec")
    chbase_c = nc.inline_tensor(
        np.broadcast_to((np.arange(CK, dtype=np.uint32) * 800)[:, None],
                        (CK, K)).copy(), "chbasec")
    zpad32_c = nc.inline_tensor(np.zeros((1, JP - J), np.uint32), "zpad32c")
    fpad_c = nc.inline_tensor(np.zeros((1, JP - J), np.float32), "fpadc")

    # DRAM scratch
    cd_d = [nc.dram_tensor(f"cd_d{i}", [136, H], F32) for i in range(IPC)]
    gd_d = [nc.dram_tensor(f"gd_d{i}", [136, H], F32) for i in range(IPC)]
    g16_d = [nc.dram_tensor(f"g16_d{i}", [CK * 800], U16) for i in range(IPC)]
    sc_d = nc.dram_tensor("sc_d", [IPC, JP], F32)            # scores bounce
    si_d = nc.dram_tensor("si_d", [IPC, JP], U16)            # gather table bounce
    so_d = nc.dram_tensor("so_d", [IPC, JP], U32)            # stage-4 offsets

    with TileContext(nc) as tc:
        with (
            tc.tile_pool(name="consts", bufs=1) as cp,
            tc.tile_pool(name="pers", bufs=1) as pp,
            tc.tile_pool(name="heat", bufs=1) as hp,
            tc.tile_pool(name="ft", bufs=2) as fp,
            tc.tile_pool(name="gt", bufs=2) as gp,
            tc.tile_pool(name="osb", bufs=4) as op,
            tc.tile_pool(name="ps", bufs=1, space="PSUM") as qp,
            tc.tile_pool(name="psc", bufs=1, space="PSUM") as qcp,
        ):
            # ---- feat stream + heat prefetches start immediately ---------
            fts = {}
            heat_tiles = {}
            heat_last = []

            # heat + shifted copies load first, as single SWDGE DMAs with the
            # reorder expressed on the DRAM side (y-major traversal -> plain
            # contiguous SBUF dest).  Their small descriptors must not queue
            # behind the bulk feat descriptors on the DMA engines.
            shift_tiles = {}
            for im in range(IPC):
                ht = hp.tile([H, CK, W], F32, tag="ht", bufs=2, name=f"ht{im}")
                heat_tiles[im] = ht
                src_ymaj = heat_in[im].rearrange("c h w -> h c w")
                d = nc.gpsimd.dma_start(out=ht, in_=src_ymaj)
                heat_last.append(d.ins)
                hu = hp.tile([H, CK, W], F32, tag="hu", bufs=2, name=f"hu{im}")
                hd = hp.tile([H, CK, W], F32, tag="hd", bufs=2, name=f"hd{im}")
                shift_tiles[im] = (hu, hd)
                nc.gpsimd.dma_start(out=hu[0:H - 1], in_=src_ymaj[1:H])
                d = nc.gpsimd.dma_start(out=hu[H - 1:H], in_=src_ymaj[H - 1:H])
                heat_last.append(d.ins)
                nc.gpsimd.dma_start(out=hd[1:H], in_=src_ymaj[0:H - 1])
                d = nc.gpsimd.dma_start(out=hd[0:1], in_=src_ymaj[0:1])
                heat_last.append(d.ins)

            def feat_load(im, t):
                ft = fts[(im, t)]
                for h in range(4):
                    d = nc.sync.dma_start(
                        out=ft[:, 3800 * h:3800 * (h + 1)],
                        in_=feat_in[im, 128 * t:128 * (t + 1),
                                    3800 * h:3800 * (h + 1)])
                    for hl in heat_last:
                        add_dep_helper(d.ins, hl, sync=True,
                                       reason="feat waits for heat stream")

            for im in range(IPC):
                for t in range(2):
                    fts[(im, t)] = fp.tile([128, HW], F32, tag="ft",
                                           name=f"ft{im}_{t}")
            for im in range(IPC):
                nc.scalar.dma_start(out=so_d[im, J:JP], in_=zpad32_c[0])
                nc.scalar.dma_start(out=sc_d[im, J:JP], in_=fpad_c[0])
            for t in range(2):
                feat_load(0, t)
            def emit_ctr():
                rt = cp.tile([125, 160, 4], F32, tag="regs", name="rt")
                nc.sync.dma_start(out=rt,
                                  in_=regs_in[:].rearrange("(p a) b -> p a b", p=125))
                l_, t_, r_, b_ = (rt[:, :, i] for i in range(4))
                mn = cp.tile([125, 160], F32, tag="ctr_mn", name="ctr_mn")
                mx = cp.tile([125, 160], F32, tag="ctr_mx", name="ctr_mx")
                pr = cp.tile([125, 160], F32, tag="ctr_pr", name="ctr_pr")
                nc.vector.tensor_tensor(out=mn, in0=l_, in1=r_, op=mybir.AluOpType.min)
                nc.vector.tensor_tensor(out=mx, in0=l_, in1=r_, op=MAX)
                nc.vector.reciprocal(out=mx, in_=mx)
                nc.vector.tensor_tensor(out=pr, in0=mn, in1=mx, op=MUL)
                nc.vector.tensor_tensor(out=mn, in0=t_, in1=b_, op=mybir.AluOpType.min)
                nc.vector.tensor_tensor(out=mx, in0=t_, in1=b_, op=MAX)
                nc.vector.reciprocal(out=mx, in_=mx)
                nc.vector.tensor_tensor(out=mn, in0=mn, in1=mx, op=MUL)
                nc.vector.tensor_tensor(out=pr, in0=pr, in1=mn, op=MUL)
                acc = cp.tile([128, 1], F32, tag="ctr_acc", name="ctr_acc")
                nc.vector.memset(acc, 0.0)
                ctr_s = cp.tile([125, 160], F32, tag="ctr_s", name="ctr_s")
                nc.scalar.activation(out=ctr_s, in_=pr,
                                     func=mybir.ActivationFunctionType.Sqrt,
                                     accum_out=acc[0:125, :])
                ones_col = cp.tile([128, 1], F32, tag="ones_col", name="ones_col")
                nc.vector.memset(ones_col, 1.0)
                ones_row = cp.tile([1, 128], F32, tag="ones_row", name="ones_row")
                nc.vector.memset(ones_row, 1.0)
                tot_p = qcp.tile([1, 1], F32, tag="aux0", name="tot_p")
                nc.tensor.matmul(tot_p, ones_col, acc, start=True, stop=True)
                mean1 = cp.tile([1, 1], F32, tag="mean1", name="mean1")
                nc.scalar.activation(out=mean1, in_=tot_p,
                                     func=mybir.ActivationFunctionType.Copy,
                                     scale=1.0 / NREG)
                mean_p = qcp.tile([128, 1], F32, tag="aux1", name="mean_p")
                nc.tensor.matmul(mean_p, ones_row, mean1, start=True, stop=True)
                mean128 = cp.tile([128, 1], F32, tag="mean128", name="mean128")
                nc.vector.tensor_copy(out=mean128, in_=mean_p)
                return mean128

            ident = cp.tile([128, 128], F32, tag="ident")
            make_identity(nc, ident)
            rowbase = cp.tile([H, CK, 8], F32, tag="rowbase")
            nc.sync.dma_start(out=rowbase, in_=rowbase_c[:].rearrange(
                "h (c s) -> h c s", s=8))
            chbase = cp.tile([CK, K], U32, tag="chbase")
            nc.sync.dma_start(out=chbase, in_=chbase_c[:])
            mean128 = None

            score_sb = []
            widx = []
            all_pts = []
            # ---------------- heat pipeline, one image at a time ----------
            for im in range(IPC):
                ht = heat_tiles[im]
                hu, hd = shift_tiles[im]
                # H-direction 3-max into hu
                nc.vector.tensor_tensor(out=hu, in0=hu, in1=ht, op=MAX)
                nc.vector.tensor_tensor(out=hu, in0=hu, in1=hd, op=MAX)
                # W-direction 3-max into hd
                nc.vector.tensor_tensor(out=hd[:, :, 0:W - 1], in0=hu[:, :, 0:W - 1],
                                        in1=hu[:, :, 1:W], op=MAX)
                nc.vector.tensor_copy(out=hd[:, :, W - 1:W], in_=hu[:, :, W - 1:W])
                nc.vector.tensor_tensor(out=hd[:, :, 1:W], in0=hd[:, :, 1:W],
                                        in1=hu[:, :, 0:W - 1], op=MAX)
                # keep only local maxima: ht *= (ht >= hd)
                nc.vector.tensor_tensor(out=hu, in0=ht, in1=hd, op=GE)
                nc.gpsimd.tensor_tensor(out=ht, in0=ht, in1=hu, op=MUL)

                # stage 1: per-y-row top-8 values + row-local indices
                vals8 = pp.tile([H, CK, 8], F32, tag="vals8", name=f"vals8_{im}")
                rl8 = pp.tile([H, CK, 8], U16, tag="rl8", name=f"rl8_{im}")
                for ch in range(CK):
                    nc.vector.max(out=vals8[:, ch, :], in_=ht[:, ch, :])
                    nc.vector.max_index(out=rl8[:, ch, :], in_max=vals8[:, ch, :],
                                        in_values=ht[:, ch, :])
                # global flat index, in f32 (exact below 2^24)
                gidxf = pp.tile([H, CK, 8], F32, tag="gidxf", name=f"gidxf_{im}")
                nc.vector.tensor_copy(out=gidxf, in_=rl8)
                nc.vector.tensor_tensor(out=gidxf, in0=gidxf, in1=rowbase, op=ADD)

                # stage 2: transpose candidates to per-channel rows via PE.
                # m-order: m = s*100 + y  (cand[ch, m] = vals8[y, ch, s])
                cand = pp.tile([CK, 800], F32, tag="cand", name=f"cand_{im}")
                g16f = pp.tile([CK, 800], F32, tag="g16f", name=f"g16f_{im}")
                for srct, dst, dscr in ((vals8, cand, cd_d[im]),
                                        (gidxf, g16f, gd_d[im])):
                    for blk in range(2):
                        p0, pn = blk * 128, min(136 - blk * 128, 128)
                        tp = qcp.tile([128, H], F32, tag=f"aux{blk}",
                                      name=f"st2_{im}_{blk}_{dst.tensor.name}")
                        nc.tensor.transpose(
                            out=tp[0:pn, :],
                            in_=srct[:].rearrange("h c s -> h (c s)")[:, p0:p0 + pn],
                            identity=ident[0:H, 0:H])
                        tps = pp.tile([128, H], F32, tag=f"st2s_{blk}",
                                      name=f"st2s_{im}_{blk}_{dst.tensor.name}")
                        nc.vector.tensor_copy(out=tps[0:pn, :], in_=tp[0:pn, :])
                        nc.scalar.dma_start(out=dscr[p0:p0 + pn, :], in_=tps[0:pn, :])
                    # dst[ch, s*100+y] = dscr[ch*8+s, y]
                    nc.scalar.dma_start(
                        out=dst,
                        in_=dscr[:].rearrange("(c s) h -> c (s h)", s=8))

                # stage 3: per-channel top-40 of the 800 candidates
                tv = pp.tile([CK, K], F32, tag=f"tv_{im}")
                cpos = pp.tile([CK, K], U32, tag=f"cpos_{im}")
                for r in range(5):
                    v8 = tv[:, r * 8:(r + 1) * 8]
                    nc.vector.max(out=v8, in_=cand)
                    nc.vector.max_index(out=cpos[:, r * 8:(r + 1) * 8], in_max=v8,
                                        in_values=cand)
                    nc.vector.match_replace(out=cand, in_to_replace=v8,
                                            in_values=cand, imm_value=-1.0)

                # stage 4: global indices.  g16f (f32) -> u16 -> DRAM
                # (contiguous), then 6 indirect row-gathers of [128, 1].
                g16u = pp.tile([CK, 800], U16, tag="g16u", name=f"g16u_{im}")
                nc.vector.tensor_copy(out=g16u, in_=g16f)
                nc.scalar.dma_start(out=g16_d[im][:].rearrange("(c m) -> c m", m=800),
                                    in_=g16u)
                offs = pp.tile([CK, K], U32, tag=f"offs_{im}")
                nc.vector.tensor_tensor(out=offs, in0=cpos, in1=chbase, op=ADD)
                nc.scalar.dma_start(out=so_d[im, 0:J].rearrange("(c k) -> c k", k=K),
                                    in_=offs)
                o128 = pp.tile([128, JP // 128], U32, tag=f"o128_{im}")
                nc.scalar.dma_start(
                    out=o128,
                    in_=AP(tensor=so_d[:].tensor, offset=im * JP,
                           ap=[[1, 128], [128, JP // 128]]))
                gi128 = pp.tile([128, JP // 128], U16, tag=f"gi128_{im}")
                for q in range(JP // 128):
                    nc.gpsimd.indirect_dma_start(
                        out=gi128[:, q:q + 1], out_offset=None,
                        in_=AP(tensor=g16_d[im][:].tensor, offset=0,
                               ap=[[1, CK * 800], [1, 1]]),
                        in_offset=bass.IndirectOffsetOnAxis(ap=o128[:, q:q + 1],
                                                            axis=0))

                # stage 5: wrapped gather table + per-tile scores
                nc.scalar.dma_start(
                    out=AP(tensor=si_d[:].tensor, offset=im * JP,
                           ap=[[1, 128], [128, JP // 128]]),
                    in_=gi128)
                wi = pp.tile([128, JP // 16], U16, tag=f"widx_{im}")
                for grp in range(8):
                    q_ = nc.scalar if grp % 2 == 0 else nc.sync
                    q_.dma_start(
                        out=wi[16 * grp:16 * (grp + 1), :],
                        in_=AP(tensor=si_d[:].tensor, offset=im * JP,
                               ap=[[1, 16], [16, JP // 16]]))
                widx.append(wi)
                nc.scalar.dma_start(out=sc_d[im, 0:J].rearrange("(c k) -> c k", k=K),
                                    in_=tv)
                ssb = pp.tile([128, JP // 128], F32, tag=f"ssb_{im}")
                nc.scalar.dma_start(
                    out=ssb,
                    in_=AP(tensor=sc_d[:].tensor, offset=im * JP,
                           ap=[[1, 128], [128, JP // 128]]))
                score_sb.append(ssb)
                if im == 0:
                    mean128 = emit_ctr()

                # ---- gather + transpose for this image ----
                pts = []
                for q in range(6):
                    pts.append(qp.tile([128, 256], F32, tag=f"pt{q}",
                                       name=f"pt{im}_{q}"))
                all_pts.append(pts)
                for t in range(2):
                    ft = fts[(im, t)]
                    if im > 0:
                        feat_load(im, t)
                    g = gp.tile([128, JP], F32, tag="g", name=f"g{im}_{t}")
                    nc.gpsimd.indirect_copy(out=g, data=ft, idxs=wi,
                                            i_know_ap_gather_is_preferred=True)
                    for q in range(6):
                        nc.tensor.transpose(out=pts[q][:, 128 * t:128 * (t + 1)],
                                            in_=g[:, 128 * q:128 * (q + 1)],
                                            identity=ident)

            # ---------------- scale/bias + output -------------------------
            for im in range(IPC):
                for q in range(6):
                    osb = op.tile([128, 256], F32, tag="osb", name=f"osb{im}_{q}")
                    nc.scalar.activation(out=osb, in_=all_pts[im][q],
                                         func=mybir.ActivationFunctionType.Identity,
                                         bias=mean128[:, :1],
                                         scale=score_sb[im][:, q:q + 1])
                    if q < 5:
                        nc.sync.dma_start(out=out_o[im, 128 * q:128 * (q + 1), :],
                                          in_=osb)
                    else:
                        nc.sync.dma_start(out=out_o[im, 640:J, :],
                                          in_=osb[0:J - 640, :])
    nc.finalize()
    return nc


def kernel(heat: np.ndarray, feat: np.ndarray, reg_targets: np.ndarray) -> np.ndarray:
    heat = np.ascontiguousarray(heat, dtype=np.float32)
    feat = np.ascontiguousarray(feat, dtype=np.float32).reshape(B, CF, HW)
    regs = np.ascontiguousarray(reg_targets, dtype=np.float32)

    nc = build_program()
    in_maps = [
        {"heat": heat[c * IPC:(c + 1) * IPC],
         "feat": feat[c * IPC:(c + 1) * IPC],
         "regs": regs}
        for c in range(NCORES)
    ]
    res = run_bass_kernel_spmd(nc, in_maps, list(range(NCORES)))
    out = np.concatenate([np.asarray(r["out"]) for r in res.results], axis=0)
    return out.reshape(B, J, CF)


if __name__ == "__main__":
    import ref_numpy as RN
    inputs = RN.get_inputs()
    exp = RN.get_expected(inputs)
    got = kernel(inputs["heat"], inputs["feat"], inputs["reg_targets"])
    err = np.abs(got - exp).max() / np.abs(exp).max()
    print("Relative error:", err)


# revision 29
# speedup vs baseline: 1.5207x; 1.0818x over previous
"""Trainium2 Bass kernel for nn_DirectPoseOutputs (loss_fn).

Reference computation:
  1) 3x3 max-pool NMS on heat [16,17,100,152]
  2) per-channel top-40 (scores + flat indices), jax.lax.top_k tie order
  3) gather feat [16,256,15200] columns at the top-40 indices -> [B,680,256]
  4) ctrness mean over reg_targets [20000,4] (scalar)
  out = gathered * scores + mean_ctrness

Sharding: data-parallel over batch, 2 images per core, 8 cores. ctrness is
computed redundantly on every core (320KB read, no collectives).

Per-core pipeline (engine assignment in brackets):
  feat stream [SP queue only]: 4x [128,15200] tiles, ~21us each.
  heat path: DRAM->DRAM transpose to y-major [ACT], load ht + y-shifted
  copies [ACT]; separable NMS: H-max from shifted loads + W-max via free-dim
  shifts [DVE], mask (is_ge, mult) on [GPSIMD]; stage-1 per-y-row top-8
  via InstMax/InstMaxIndex [DVE]; stage-2 relayout to per-channel rows via
  PE transpose + coarse-grained fold DMA (m = s*100+y order; validated safe
  for the duplicate-value tie cases); stage-3 top-40 via 5 rounds of
  max/max_index/match_replace [DVE]; stage-4 global indices via 6 indirect
  DMAs of [128,1] (one offset per dest partition row -- HW semantics);
  feat gather via GPSIMD indirect_copy (shared wrapped index table);
  PE-transpose [c,j]->[j,c]; fused out = psum*score[j] + mean [ACT]; out
  DMAs [SP].
"""
import numpy as np

import concourse.bass as bass
import concourse.bacc as bacc
import concourse.mybir as mybir
from concourse.bass_types import AP
from concourse.bass_utils import run_bass_kernel_spmd
from concourse.masks import make_identity
from concourse.tile import TileContext, add_dep_helper

F32 = mybir.dt.float32
U16 = mybir.dt.uint16
U32 = mybir.dt.uint32

B, CK, H, W = 16, 17, 100, 152
HW = H * W            # 15200
CW = CK * W           # 2584 free width of the y-major heat tile
CF = 256
K = 40
NCORES = 8
IPC = B // NCORES     # images per core = 2
J = CK * K            # 680 output rows per image
JP = 768              # padded to 6 tiles of 128 rows (and %16 for the table)
NREG = 20000
MAX, GE, MUL, ADD = (mybir.AluOpType.max, mybir.AluOpType.is_ge,
                     mybir.AluOpType.mult, mybir.AluOpType.add)


def build_program() -> bass.Bass:
    nc = bacc.Bacc()

    heat_in = nc.declare_dram_parameter("heat", [IPC, CK, H, W], F32, isOutput=False)
    feat_in = nc.declare_dram_parameter("feat", [IPC, CF, HW], F32, isOutput=False)
    regs_in = nc.declare_dram_parameter("regs", [NREG, 4], F32, isOutput=False)
    out_o = nc.declare_dram_parameter("out", [IPC, J, CF], F32, isOutput=True)

    # constants
    rowbase_c = nc.inline_tensor(
        np.broadcast_to((np.arange(H, dtype=np.float32) * W)[:, None],
                        (H, CK * 8)).copy(), "rowbasec")
    chbase_c = nc.inline_tensor(
        np.broadcast_to((np.arange(CK, dtype=np.uint32) * 800)[:, None],
                        (CK, K)).copy(), "chbasec")
    zpad32_c = nc.inline_tensor(np.zeros((1, JP - J), np.uint32), "zpad32c")
    fpad_c = nc.inline_tensor(np.zeros((1, JP - J), np.float32), "fpadc")

    # DRAM scratch
    cd_d = [nc.dram_tensor(f"cd_d{i}", [136, H], F32) for i in range(IPC)]
    gd_d = [nc.dram_tensor(f"gd_d{i}", [136, H], F32) for i in range(IPC)]
    g16_d = [nc.dram_tensor(f"g16_d{i}", [CK * 800], U16) for i in range(IPC)]
    sc_d = nc.dram_tensor("sc_d", [IPC, JP], F32)            # scores bounce
    si_d = nc.dram_tensor("si_d", [IPC, JP], U16)            # gather table bounce
    so_d = nc.dram_tensor("so_d", [IPC, JP], U32)            # stage-4 offsets

    with TileContext(nc) as tc:
        with (
            tc.tile_pool(name="consts", bufs=1) as cp,
            tc.tile_pool(name="pers", bufs=1) as pp,
            tc.tile_pool(name="heat", bufs=1) as hp,
            tc.tile_pool(name="ft", bufs=2) as fp,
            tc.tile_pool(name="gt", bufs=2) as gp,
            tc.tile_pool(name="osb", bufs=4) as op,
            tc.tile_pool(name="ps", bufs=1, space="PSUM") as qp,
            tc.tile_pool(name="psc", bufs=1, space="PSUM") as qcp,
        ):
            # ---- feat stream + heat prefetches start immediately ---------
            fts = {}
            heat_tiles = {}
            heat_last = []

            # heat + shifted copies load first, as single SWDGE DMAs with the
            # reorder expressed on the DRAM side (y-major traversal -> plain
            # contiguous SBUF dest).  Their small descriptors must not queue
            # behind the bulk feat descriptors on the DMA engines.
            shift_tiles = {}
            for im in range(IPC):
                ht = hp.tile([H, CK, W], F32, tag="ht", bufs=2, name=f"ht{im}")
                heat_tiles[im] = ht
                src_ymaj = heat_in[im].rearrange("c h w -> h c w")
                d = nc.gpsimd.dma_start(out=ht, in_=src_ymaj)
                heat_last.append(d.ins)
                hu = hp.tile([H, CK, W], F32, tag="hu", bufs=2, name=f"hu{im}")
                hd = hp.tile([H, CK, W], F32, tag="hd", bufs=2, name=f"hd{im}")
                shift_tiles[im] = (hu, hd)
                nc.gpsimd.dma_start(out=hu[0:H - 1], in_=src_ymaj[1:H])
                d = nc.gpsimd.dma_start(out=hu[H - 1:H], in_=src_ymaj[H - 1:H])
                heat_last.append(d.ins)
                nc.gpsimd.dma_start(out=hd[1:H], in_=src_ymaj[0:H - 1])
                d = nc.gpsimd.dma_start(out=hd[0:1], in_=src_ymaj[0:1])
                heat_last.append(d.ins)

            def feat_load(im, t):
                ft = fts[(im, t)]
                for h in range(4):
                    d = nc.sync.dma_start(
                        out=ft[:, 3800 * h:3800 * (h + 1)],
                        in_=feat_in[im, 128 * t:128 * (t + 1),
                                    3800 * h:3800 * (h + 1)])
                    for hl in heat_last:
                        add_dep_helper(d.ins, hl, sync=True,
                                       reason="feat waits for heat stream")

            for im in range(IPC):
                for t in range(2):
                    fts[(im, t)] = fp.tile([128, HW], F32, tag="ft",
                                           name=f"ft{im}_{t}")
            for im in range(IPC):
                nc.scalar.dma_start(out=so_d[im, J:JP], in_=zpad32_c[0])
                nc.scalar.dma_start(out=sc_d[im, J:JP], in_=fpad_c[0])
            for t in range(2):
                feat_load(0, t)
            def emit_ctr():
                rt = cp.tile([125, 160, 4], F32, tag="regs", name="rt")
                nc.sync.dma_start(out=rt,
                                  in_=regs_in[:].rearrange("(p a) b -> p a b", p=125))
                l_, t_, r_, b_ = (rt[:, :, i] for i in range(4))
                mn = cp.tile([125, 160], F32, tag="ctr_mn", name="ctr_mn")
                mx = cp.tile([125, 160], F32, tag="ctr_mx", name="ctr_mx")
                pr = cp.tile([125, 160], F32, tag="ctr_pr", name="ctr_pr")
                nc.vector.tensor_tensor(out=mn, in0=l_, in1=r_, op=mybir.AluOpType.min)
                nc.vector.tensor_tensor(out=mx, in0=l_, in1=r_, op=MAX)
                nc.vector.reciprocal(out=mx, in_=mx)
                nc.vector.tensor_tensor(out=pr, in0=mn, in1=mx, op=MUL)
                nc.vector.tensor_tensor(out=mn, in0=t_, in1=b_, op=mybir.AluOpType.min)
                nc.vector.tensor_tensor(out=mx, in0=t_, in1=b_, op=MAX)
                nc.vector.reciprocal(out=mx, in_=mx)
                nc.vector.tensor_tensor(out=mn, in0=mn, in1=mx, op=MUL)
                nc.vector.tensor_tensor(out=pr, in0=pr, in1=mn, op=MUL)
                acc = cp.tile([128, 1], F32, tag="ctr_acc", name="ctr_acc")
                nc.vector.memset(acc, 0.0)
                ctr_s = cp.tile([125, 160], F32, tag="ctr_s", name="ctr_s")
                nc.scalar.activation(out=ctr_s, in_=pr,
                                     func=mybir.ActivationFunctionType.Sqrt,
                                     accum_out=acc[0:125, :])
                ones_col = cp.tile([128, 1], F32, tag="ones_col", name="ones_col")
                nc.vector.memset(ones_col, 1.0)
                ones_row = cp.tile([1, 128], F32, tag="ones_row", name="ones_row")
                nc.vector.memset(ones_row, 1.0)
                tot_p = qcp.tile([1, 1], F32, tag="aux0", name="tot_p")
                nc.tensor.matmul(tot_p, ones_col, acc, start=True, stop=True)
                mean1 = cp.tile([1, 1], F32, tag="mean1", name="mean1")
                nc.scalar.activation(out=mean1, in_=tot_p,
                                     func=mybir.ActivationFunctionType.Copy,
                                     scale=1.0 / NREG)
                mean_p = qcp.tile([128, 1], F32, tag="aux1", name="mean_p")
                nc.tensor.matmul(mean_p, ones_row, mean1, start=True, stop=True)
                mean128 = cp.tile([128, 1], F32, tag="mean128", name="mean128")
                nc.vector.tensor_copy(out=mean128, in_=mean_p)
                return mean128

            ident = cp.tile([128, 128], F32, tag="ident")
            make_identity(nc, ident)
            rowbase = cp.tile([H, CK, 8], F32, tag="rowbase")
            nc.sync.dma_start(out=rowbase, in_=rowbase_c[:].rearrange(
                "h (c s) -> h c s", s=8))
            chbase = cp.tile([CK, K], U32, tag="chbase")
            nc.sync.dma_start(out=chbase, in_=chbase_c[:])
            mean128 = None

            score_sb = []
            widx = []
            all_pts = []
            # ---------------- heat pipeline, one image at a time ----------
            for im in range(IPC):
                ht = heat_tiles[im]
                hu, hd = shift_tiles[im]
                # H-direction 3-max into hu
                nc.vector.tensor_tensor(out=hu, in0=hu, in1=ht, op=MAX)
                nc.vector.tensor_tensor(out=hu, in0=hu, in1=hd, op=MAX)
                # W-direction 3-max into hd
                nc.vector.tensor_tensor(out=hd[:, :, 0:W - 1], in0=hu[:, :, 0:W - 1],
                                        in1=hu[:, :, 1:W], op=MAX)
                nc.vector.tensor_copy(out=hd[:, :, W - 1:W], in_=hu[:, :, W - 1:W])
                nc.vector.tensor_tensor(out=hd[:, :, 1:W], in0=hd[:, :, 1:W],
                                        in1=hu[:, :, 0:W - 1], op=MAX)
                # keep only local maxima: ht *= (ht >= hd)
                nc.vector.tensor_tensor(out=hu, in0=ht, in1=hd, op=GE)
                nc.gpsimd.tensor_tensor(out=ht, in0=ht, in1=hu, op=MUL)

                # stage 1: per-y-row top-8 values + row-local indices
                vals8 = pp.tile([H, CK, 8], F32, tag="vals8", name=f"vals8_{im}")
                rl8 = pp.tile([H, CK, 8], U16, tag="rl8", name=f"rl8_{im}")
                for ch in range(CK):
                    nc.vector.max(out=vals8[:, ch, :], in_=ht[:, ch, :])
                    nc.vector.max_index(out=rl8[:, ch, :], in_max=vals8[:, ch, :],
                                        in_values=ht[:, ch, :])
                # global flat index, in f32 (exact below 2^24)
                gidxf = pp.tile([H, CK, 8], F32, tag="gidxf", name=f"gidxf_{im}")
                nc.vector.tensor_copy(out=gidxf, in_=rl8)
                nc.vector.tensor_tensor(out=gidxf, in0=gidxf, in1=rowbase, op=ADD)

                # stage 2: transpose candidates to per-channel rows via PE.
                # m-order: m = s*100 + y  (cand[ch, m] = vals8[y, ch, s])
                cand = pp.tile([CK, 800], F32, tag="cand", name=f"cand_{im}")
                g16f = pp.tile([CK, 800], F32, tag="g16f", name=f"g16f_{im}")
                for srct, dst, dscr in ((vals8, cand, cd_d[im]),
                                        (gidxf, g16f, gd_d[im])):
                    for blk in range(2):
                        p0, pn = blk * 128, min(136 - blk * 128, 128)
                        c0, cn = blk * 16, (16 if blk == 0 else 1)
                        tp = qcp.tile([128, H], F32, tag=f"aux{blk}",
                                      name=f"st2_{im}_{blk}_{dst.tensor.name}")
                        nc.tensor.transpose(
                            out=tp[0:pn, :],
                            in_=srct[:].rearrange("h c s -> h (c s)")[:, p0:p0 + pn],
                            identity=ident[0:H, 0:H])
                        tps = pp.tile([128, H], F32, tag=f"st2s_{blk}",
                                      name=f"st2s_{im}_{blk}_{dst.tensor.name}")
                        nc.vector.tensor_copy(out=tps[0:pn, :], in_=tp[0:pn, :])
                        nc.scalar.dma_start(out=dscr[p0:p0 + pn, :], in_=tps[0:pn, :])
                    nc.scalar.dma_start(
                        out=dst,
                        in_=dscr[:].rearrange("(c s) h -> c (s h)", s=8))

                # stage 3: per-channel top-40 of the 800 candidates
                tv = pp.tile([CK, K], F32, tag=f"tv_{im}")
                cpos = pp.tile([CK, K], U32, tag=f"cpos_{im}")
                for r in range(5):
                    v8 = tv[:, r * 8:(r + 1) * 8]
                    nc.vector.max(out=v8, in_=cand)
                    nc.vector.max_index(out=cpos[:, r * 8:(r + 1) * 8], in_max=v8,
                                        in_values=cand)
                    nc.vector.match_replace(out=cand, in_to_replace=v8,
                                            in_values=cand, imm_value=-1.0)

                # stage 4: global indices.  g16f (f32) -> u16 -> DRAM
                # (contiguous), then 6 indirect row-gathers of [128, 1].
                g16u = pp.tile([CK, 800], U16, tag="g16u", name=f"g16u_{im}")
                nc.vector.tensor_copy(out=g16u, in_=g16f)
                nc.scalar.dma_start(out=g16_d[im][:].rearrange("(c m) -> c m", m=800),
                                    in_=g16u)
                offs = pp.tile([CK, K], U32, tag=f"offs_{im}")
                nc.vector.tensor_tensor(out=offs, in0=cpos, in1=chbase, op=ADD)
                nc.scalar.dma_start(out=so_d[im, 0:J].rearrange("(c k) -> c k", k=K),
                                    in_=offs)
                o128 = pp.tile([128, JP // 128], U32, tag=f"o128_{im}")
                nc.scalar.dma_start(
                    out=o128,
                    in_=AP(tensor=so_d[:].tensor, offset=im * JP,
                           ap=[[1, 128], [128, JP // 128]]))
                gi128 = pp.tile([128, JP // 128], U16, tag=f"gi128_{im}")
                for q in range(JP // 128):
                    nc.gpsimd.indirect_dma_start(
                        out=gi128[:, q:q + 1], out_offset=None,
                        in_=AP(tensor=g16_d[im][:].tensor, offset=0,
                               ap=[[1, CK * 800], [1, 1]]),
                        in_offset=bass.IndirectOffsetOnAxis(ap=o128[:, q:q + 1],
                                                            axis=0))

                # stage 5: wrapped gather table + per-tile scores
                nc.scalar.dma_start(
                    out=AP(tensor=si_d[:].tensor, offset=im * JP,
                           ap=[[1, 128], [128, JP // 128]]),
                    in_=gi128)
                wi = pp.tile([128, JP // 16], U16, tag=f"widx_{im}")
                for grp in range(8):
                    q_ = (nc.scalar, nc.sync, nc.gpsimd)[grp % 3]
                    q_.dma_start(
                        out=wi[16 * grp:16 * (grp + 1), :],
                        in_=AP(tensor=si_d[:].tensor, offset=im * JP,
                               ap=[[1, 16], [16, JP // 16]]))
                widx.append(wi)
                nc.scalar.dma_start(out=sc_d[im, 0:J].rearrange("(c k) -> c k", k=K),
                                    in_=tv)
                ssb = pp.tile([128, JP // 128], F32, tag=f"ssb_{im}")
                nc.scalar.dma_start(
                    out=ssb,
                    in_=AP(tensor=sc_d[:].tensor, offset=im * JP,
                           ap=[[1, 128], [128, JP // 128]]))
                score_sb.append(ssb)
                if im == 0:
                    mean128 = emit_ctr()

                # ---- gather + transpose for this image ----
                pts = []
                for q in range(6):
                    pts.append(qp.tile([128, 256], F32, tag=f"pt{q}",
                                       name=f"pt{im}_{q}"))
                all_pts.append(pts)
                for t in range(2):
                    ft = fts[(im, t)]
                    if im > 0:
                        feat_load(im, t)
                    g = gp.tile([128, JP], F32, tag="g", name=f"g{im}_{t}")
                    nc.gpsimd.indirect_copy(out=g, data=ft, idxs=wi,
                                            i_know_ap_gather_is_preferred=True)
                    for q in range(6):
                        nc.tensor.transpose(out=pts[q][:, 128 * t:128 * (t + 1)],
                                            in_=g[:, 128 * q:128 * (q + 1)],
                                            identity=ident)

            # ---------------- scale/bias + output -------------------------
            for im in range(IPC):
                for q in range(6):
                    osb = op.tile([128, 256], F32, tag="osb", name=f"osb{im}_{q}")
                    nc.scalar.activation(out=osb, in_=all_pts[im][q],
                                         func=mybir.ActivationFunctionType.Identity,
                                         bias=mean128[:, :1],
                                         scale=score_sb[im][:, q:q + 1])
                    if q < 5:
                        nc.sync.dma_start(out=out_o[im, 128 * q:128 * (q + 1), :],
                                          in_=osb)
                    else:
                        nc.sync.dma_start(out=out_o[im, 640:J, :],
                                          in_=osb[0:J - 640, :])
    nc.finalize()
    return nc


def kernel(heat: np.ndarray, feat: np.ndarray, reg_targets: np.ndarray) -> np.ndarray:
    heat = np.ascontiguousarray(heat, dtype=np.float32)
    feat = np.ascontiguousarray(feat, dtype=np.float32).reshape(B, CF, HW)
    regs = np.ascontiguousarray(reg_targets, dtype=np.float32)

    nc = build_program()
    in_maps = [
        {"heat": heat[c * IPC:(c + 1) * IPC],
         "feat": feat[c * IPC:(c + 1) * IPC],
         "regs": regs}
        for c in range(NCORES)
    ]
    res = run_bass_kernel_spmd(nc, in_maps, list(range(NCORES)))
    out = np.concatenate([np.asarray(r["out"]) for r in res.results], axis=0)
    return out.reshape(B, J, CF)


if __name__ == "__main__":
    import ref_numpy as RN
    inputs = RN.get_inputs()
    exp = RN.get_expected(inputs)
    got = kernel(inputs["heat"], inputs["feat"], inputs["reg_targets"])
    err = np.abs(got - exp).max() / np.abs(exp).max()
    print("Relative error:", err)


# revision 30
# speedup vs baseline: 1.5255x; 1.0031x over previous
"""Trainium2 Bass kernel for nn_DirectPoseOutputs (loss_fn).

Reference computation:
  1) 3x3 max-pool NMS on heat [16,17,100,152]
  2) per-channel top-40 (scores + flat indices), jax.lax.top_k tie order
  3) gather feat [16,256,15200] columns at the top-40 indices -> [B,680,256]
  4) ctrness mean over reg_targets [20000,4] (scalar)
  out = gathered * scores + mean_ctrness

Sharding: data-parallel over batch, 2 images per core, 8 cores. ctrness is
computed redundantly on every core (320KB read, no collectives).

Per-core pipeline (engine assignment in brackets):
  feat stream [SP queue only]: 4x [128,15200] tiles, ~21us each.
  heat path: DRAM->DRAM transpose to y-major [ACT], load ht + y-shifted
  copies [ACT]; separable NMS: H-max from shifted loads + W-max via free-dim
  shifts [DVE], mask (is_ge, mult) on [GPSIMD]; stage-1 per-y-row top-8
  via InstMax/InstMaxIndex [DVE]; stage-2 relayout to per-channel rows via
  PE transpose + coarse-grained fold DMA (m = s*100+y order; validated safe
  for the duplicate-value tie cases); stage-3 top-40 via 5 rounds of
  max/max_index/match_replace [DVE]; stage-4 global indices via 6 indirect
  DMAs of [128,1] (one offset per dest partition row -- HW semantics);
  feat gather via GPSIMD indirect_copy (shared wrapped index table);
  PE-transpose [c,j]->[j,c]; fused out = psum*score[j] + mean [ACT]; out
  DMAs [SP].
"""
import numpy as np

import concourse.bass as bass
import concourse.bacc as bacc
import concourse.mybir as mybir
from concourse.bass_types import AP
from concourse.bass_utils import run_bass_kernel_spmd
from concourse.masks import make_identity
from concourse.tile import TileContext, add_dep_helper

F32 = mybir.dt.float32
U16 = mybir.dt.uint16
U32 = mybir.dt.uint32

B, CK, H, W = 16, 17, 100, 152
HW = H * W            # 15200
CW = CK * W           # 2584 free width of the y-major heat tile
CF = 256
K = 40
NCORES = 8
IPC = B // NCORES     # images per core = 2
J = CK * K            # 680 output rows per image
JP = 768              # padded to 6 tiles of 128 rows (and %16 for the table)
NREG = 20000
MAX, GE, MUL, ADD = (mybir.AluOpType.max, mybir.AluOpType.is_ge,
                     mybir.AluOpType.mult, mybir.AluOpType.add)


def build_program() -> bass.Bass:
    nc = bacc.Bacc()

    heat_in = nc.declare_dram_parameter("heat", [IPC, CK, H, W], F32, isOutput=False)
    feat_in = nc.declare_dram_parameter("feat", [IPC, CF, HW], F32, isOutput=False)
    regs_in = nc.declare_dram_parameter("regs", [NREG, 4], F32, isOutput=False)
    out_o = nc.declare_dram_parameter("out", [IPC, J, CF], F32, isOutput=True)

    # constants
    rowbase_c = nc.inline_tensor(
        np.broadcast_to((np.arange(H, dtype=np.float32) * W)[:, None],
                        (H, CK * 8)).copy(), "rowbasec")
    chbase_c = nc.inline_tensor(
        np.broadcast_to((np.arange(CK, dtype=np.uint32) * 800)[:, None],
                        (CK, K)).copy(), "chbasec")
    zpad32_c = nc.inline_tensor(np.zeros((1, JP - J), np.uint32), "zpad32c")
    fpad_c = nc.inline_tensor(np.zeros((1, JP - J), np.float32), "fpadc")

    # DRAM scratch
    cd_d = [nc.dram_tensor(f"cd_d{i}", [136, H], F32) for i in range(IPC)]
    gd_d = [nc.dram_tensor(f"gd_d{i}", [136, H], F32) for i in range(IPC)]
    g16_d = [nc.dram_tensor(f"g16_d{i}", [CK * 800], U16) for i in range(IPC)]
    sc_d = nc.dram_tensor("sc_d", [IPC, JP], F32)            # scores bounce
    si_d = nc.dram_tensor("si_d", [IPC, JP], U16)            # gather table bounce
    so_d = nc.dram_tensor("so_d", [IPC, JP], U32)            # stage-4 offsets

    with TileContext(nc) as tc:
        with (
            tc.tile_pool(name="consts", bufs=1) as cp,
            tc.tile_pool(name="pers", bufs=1) as pp,
            tc.tile_pool(name="heat", bufs=1) as hp,
            tc.tile_pool(name="ft", bufs=2) as fp,
            tc.tile_pool(name="gt", bufs=2) as gp,
            tc.tile_pool(name="osb", bufs=4) as op,
            tc.tile_pool(name="ps", bufs=1, space="PSUM") as qp,
            tc.tile_pool(name="psc", bufs=1, space="PSUM") as qcp,
        ):
            # ---- feat stream + heat prefetches start immediately ---------
            fts = {}
            heat_tiles = {}
            heat_last = []

            # heat + shifted copies load first, as single SWDGE DMAs with the
            # reorder expressed on the DRAM side (y-major traversal -> plain
            # contiguous SBUF dest).  Their small descriptors must not queue
            # behind the bulk feat descriptors on the DMA engines.
            shift_tiles = {}
            for im in range(IPC):
                ht = hp.tile([H, CK, W], F32, tag="ht", bufs=2, name=f"ht{im}")
                heat_tiles[im] = ht
                src_ymaj = heat_in[im].rearrange("c h w -> h c w")
                d = nc.gpsimd.dma_start(out=ht, in_=src_ymaj)
                heat_last.append(d.ins)
                hu = hp.tile([H, CK, W], F32, tag="hu", bufs=2, name=f"hu{im}")
                hd = hp.tile([H, CK, W], F32, tag="hd", bufs=2, name=f"hd{im}")
                shift_tiles[im] = (hu, hd)
                nc.gpsimd.dma_start(out=hu[0:H - 1], in_=src_ymaj[1:H])
                d = nc.gpsimd.dma_start(out=hu[H - 1:H], in_=src_ymaj[H - 1:H])
                heat_last.append(d.ins)
                nc.gpsimd.dma_start(out=hd[1:H], in_=src_ymaj[0:H - 1])
                d = nc.gpsimd.dma_start(out=hd[0:1], in_=src_ymaj[0:1])
                heat_last.append(d.ins)

            def feat_load(im, t):
                ft = fts[(im, t)]
                for h in range(4):
                    d = nc.sync.dma_start(
                        out=ft[:, 3800 * h:3800 * (h + 1)],
                        in_=feat_in[im, 128 * t:128 * (t + 1),
                                    3800 * h:3800 * (h + 1)])
                    for hl in heat_last:
                        add_dep_helper(d.ins, hl, sync=True,
                                       reason="feat waits for heat stream")

            for im in range(IPC):
                for t in range(2):
                    fts[(im, t)] = fp.tile([128, HW], F32, tag="ft",
                                           name=f"ft{im}_{t}")
            for im in range(IPC):
                nc.scalar.dma_start(out=so_d[im, J:JP], in_=zpad32_c[0])
                nc.scalar.dma_start(out=sc_d[im, J:JP], in_=fpad_c[0])
            for t in range(2):
                feat_load(0, t)
            def emit_ctr():
                rt = cp.tile([125, 160, 4], F32, tag="regs", name="rt")
                nc.sync.dma_start(out=rt,
                                  in_=regs_in[:].rearrange("(p a) b -> p a b", p=125))
                l_, t_, r_, b_ = (rt[:, :, i] for i in range(4))
                mn = cp.tile([125, 160], F32, tag="ctr_mn", name="ctr_mn")
                mx = cp.tile([125, 160], F32, tag="ctr_mx", name="ctr_mx")
                pr = cp.tile([125, 160], F32, tag="ctr_pr", name="ctr_pr")
                nc.vector.tensor_tensor(out=mn, in0=l_, in1=r_, op=mybir.AluOpType.min)
                nc.vector.tensor_tensor(out=mx, in0=l_, in1=r_, op=MAX)
                nc.vector.reciprocal(out=mx, in_=mx)
                nc.vector.tensor_tensor(out=pr, in0=mn, in1=mx, op=MUL)
                nc.vector.tensor_tensor(out=mn, in0=t_, in1=b_, op=mybir.AluOpType.min)
                nc.vector.tensor_tensor(out=mx, in0=t_, in1=b_, op=MAX)
                nc.vector.reciprocal(out=mx, in_=mx)
                nc.vector.tensor_tensor(out=mn, in0=mn, in1=mx, op=MUL)
                nc.vector.tensor_tensor(out=pr, in0=pr, in1=mn, op=MUL)
                acc = cp.tile([128, 1], F32, tag="ctr_acc", name="ctr_acc")
                nc.vector.memset(acc, 0.0)
                ctr_s = cp.tile([125, 160], F32, tag="ctr_s", name="ctr_s")
                nc.scalar.activation(out=ctr_s, in_=pr,
                                     func=mybir.ActivationFunctionType.Sqrt,
                                     accum_out=acc[0:125, :])
                ones_col = cp.tile([128, 1], F32, tag="ones_col", name="ones_col")
                nc.vector.memset(ones_col, 1.0)
                ones_row = cp.tile([1, 128], F32, tag="ones_row", name="ones_row")
                nc.vector.memset(ones_row, 1.0)
                tot_p = qcp.tile([1, 1], F32, tag="aux0", name="tot_p")
                nc.tensor.matmul(tot_p, ones_col, acc, start=True, stop=True)
                mean1 = cp.tile([1, 1], F32, tag="mean1", name="mean1")
                nc.scalar.activation(out=mean1, in_=tot_p,
                                     func=mybir.ActivationFunctionType.Copy,
                                     scale=1.0 / NREG)
                mean_p = qcp.tile([128, 1], F32, tag="aux1", name="mean_p")
                nc.tensor.matmul(mean_p, ones_row, mean1, start=True, stop=True)
                mean128 = cp.tile([128, 1], F32, tag="mean128", name="mean128")
                nc.vector.tensor_copy(out=mean128, in_=mean_p)
                return mean128

            ident = cp.tile([128, 128], F32, tag="ident")
            make_identity(nc, ident)
            rowbase = cp.tile([H, CK, 8], F32, tag="rowbase")
            nc.sync.dma_start(out=rowbase, in_=rowbase_c[:].rearrange(
                "h (c s) -> h c s", s=8))
            chbase = cp.tile([CK, K], U32, tag="chbase")
            nc.sync.dma_start(out=chbase, in_=chbase_c[:])
            mean128 = None

            score_sb = []
            widx = []
            all_pts = []
            # ---------------- heat pipeline, one image at a time ----------
            for im in range(IPC):
                tq = nc.scalar if im == 0 else nc.sync
                ht = heat_tiles[im]
                hu, hd = shift_tiles[im]
                # H-direction 3-max into hu
                nc.vector.tensor_tensor(out=hu, in0=hu, in1=ht, op=MAX)
                nc.vector.tensor_tensor(out=hu, in0=hu, in1=hd, op=MAX)
                # W-direction 3-max into hd
                nc.vector.tensor_tensor(out=hd[:, :, 0:W - 1], in0=hu[:, :, 0:W - 1],
                                        in1=hu[:, :, 1:W], op=MAX)
                nc.vector.tensor_copy(out=hd[:, :, W - 1:W], in_=hu[:, :, W - 1:W])
                nc.vector.tensor_tensor(out=hd[:, :, 1:W], in0=hd[:, :, 1:W],
                                        in1=hu[:, :, 0:W - 1], op=MAX)
                # keep only local maxima: ht *= (ht >= hd)
                nc.vector.tensor_tensor(out=hu, in0=ht, in1=hd, op=GE)
                nc.gpsimd.tensor_tensor(out=ht, in0=ht, in1=hu, op=MUL)

                # stage 1: per-y-row top-8 values + row-local indices
                vals8 = pp.tile([H, CK, 8], F32, tag="vals8", name=f"vals8_{im}")
                rl8 = pp.tile([H, CK, 8], U16, tag="rl8", name=f"rl8_{im}")
                for ch in range(CK):
                    nc.vector.max(out=vals8[:, ch, :], in_=ht[:, ch, :])
                    nc.vector.max_index(out=rl8[:, ch, :], in_max=vals8[:, ch, :],
                                        in_values=ht[:, ch, :])
                # global flat index, in f32 (exact below 2^24)
                gidxf = pp.tile([H, CK, 8], F32, tag="gidxf", name=f"gidxf_{im}")
                nc.vector.tensor_copy(out=gidxf, in_=rl8)
                nc.vector.tensor_tensor(out=gidxf, in0=gidxf, in1=rowbase, op=ADD)

                # stage 2: transpose candidates to per-channel rows via PE.
                # m-order: m = s*100 + y  (cand[ch, m] = vals8[y, ch, s])
                cand = pp.tile([CK, 800], F32, tag="cand", name=f"cand_{im}")
                g16f = pp.tile([CK, 800], F32, tag="g16f", name=f"g16f_{im}")
                for srct, dst, dscr in ((vals8, cand, cd_d[im]),
                                        (gidxf, g16f, gd_d[im])):
                    for blk in range(2):
                        p0, pn = blk * 128, min(136 - blk * 128, 128)
                        c0, cn = blk * 16, (16 if blk == 0 else 1)
                        tp = qcp.tile([128, H], F32, tag=f"aux{blk}",
                                      name=f"st2_{im}_{blk}_{dst.tensor.name}")
                        nc.tensor.transpose(
                            out=tp[0:pn, :],
                            in_=srct[:].rearrange("h c s -> h (c s)")[:, p0:p0 + pn],
                            identity=ident[0:H, 0:H])
                        tps = pp.tile([128, H], F32, tag=f"st2s_{blk}",
                                      name=f"st2s_{im}_{blk}_{dst.tensor.name}")
                        nc.vector.tensor_copy(out=tps[0:pn, :], in_=tp[0:pn, :])
                        tq.dma_start(out=dscr[p0:p0 + pn, :], in_=tps[0:pn, :])
                    tq.dma_start(
                        out=dst,
                        in_=dscr[:].rearrange("(c s) h -> c (s h)", s=8))

                # stage 3: per-channel top-40 of the 800 candidates
                tv = pp.tile([CK, K], F32, tag=f"tv_{im}")
                cpos = pp.tile([CK, K], U32, tag=f"cpos_{im}")
                for r in range(5):
                    v8 = tv[:, r * 8:(r + 1) * 8]
                    nc.vector.max(out=v8, in_=cand)
                    nc.vector.max_index(out=cpos[:, r * 8:(r + 1) * 8], in_max=v8,
                                        in_values=cand)
                    nc.vector.match_replace(out=cand, in_to_replace=v8,
                                            in_values=cand, imm_value=-1.0)

                # stage 4: global indices.  g16f (f32) -> u16 -> DRAM
                # (contiguous), then 6 indirect row-gathers of [128, 1].
                g16u = pp.tile([CK, 800], U16, tag="g16u", name=f"g16u_{im}")
                nc.vector.tensor_copy(out=g16u, in_=g16f)
                tq.dma_start(out=g16_d[im][:].rearrange("(c m) -> c m", m=800),
                             in_=g16u)
                offs = pp.tile([CK, K], U32, tag=f"offs_{im}")
                nc.vector.tensor_tensor(out=offs, in0=cpos, in1=chbase, op=ADD)
                tq.dma_start(out=so_d[im, 0:J].rearrange("(c k) -> c k", k=K),
                             in_=offs)
                o128 = pp.tile([128, JP // 128], U32, tag=f"o128_{im}")
                tq.dma_start(
                    out=o128,
                    in_=AP(tensor=so_d[:].tensor, offset=im * JP,
                           ap=[[1, 128], [128, JP // 128]]))
                gi128 = pp.tile([128, JP // 128], U16, tag=f"gi128_{im}")
                for q in range(JP // 128):
                    nc.gpsimd.indirect_dma_start(
                        out=gi128[:, q:q + 1], out_offset=None,
                        in_=AP(tensor=g16_d[im][:].tensor, offset=0,
                               ap=[[1, CK * 800], [1, 1]]),
                        in_offset=bass.IndirectOffsetOnAxis(ap=o128[:, q:q + 1],
                                                            axis=0))

                # stage 5: wrapped gather table + per-tile scores
                tq.dma_start(
                    out=AP(tensor=si_d[:].tensor, offset=im * JP,
                           ap=[[1, 128], [128, JP // 128]]),
                    in_=gi128)
                wi = pp.tile([128, JP // 16], U16, tag=f"widx_{im}")
                for grp in range(8):
                    q_ = tq
                    q_.dma_start(
                        out=wi[16 * grp:16 * (grp + 1), :],
                        in_=AP(tensor=si_d[:].tensor, offset=im * JP,
                               ap=[[1, 16], [16, JP // 16]]))
                widx.append(wi)
                tq.dma_start(out=sc_d[im, 0:J].rearrange("(c k) -> c k", k=K),
                             in_=tv)
                ssb = pp.tile([128, JP // 128], F32, tag=f"ssb_{im}")
                tq.dma_start(
                    out=ssb,
                    in_=AP(tensor=sc_d[:].tensor, offset=im * JP,
                           ap=[[1, 128], [128, JP // 128]]))
                score_sb.append(ssb)
                if im == 0:
                    mean128 = emit_ctr()

                # ---- gather + transpose for this image ----
                pts = []
                for q in range(6):
                    pts.append(qp.tile([128, 256], F32, tag=f"pt{q}",
                                       name=f"pt{im}_{q}"))
                all_pts.append(pts)
                for t in range(2):
                    ft = fts[(im, t)]
                    if im > 0:
                        feat_load(im, t)
                    g = gp.tile([128, JP], F32, tag="g", name=f"g{im}_{t}")
                    nc.gpsimd.indirect_copy(out=g, data=ft, idxs=wi,
                                            i_know_ap_gather_is_preferred=True)
                    for q in range(6):
                        nc.tensor.transpose(out=pts[q][:, 128 * t:128 * (t + 1)],
                                            in_=g[:, 128 * q:128 * (q + 1)],
                                            identity=ident)

            # ---------------- scale/bias + output -------------------------
            for im in range(IPC):
                for q in range(6):
                    osb = op.tile([128, 256], F32, tag="osb", name=f"osb{im}_{q}")
                    nc.scalar.activation(out=osb, in_=all_pts[im][q],
                                         func=mybir.ActivationFunctionType.Identity,
                                         bias=mean128[:, :1],
                                         scale=score_sb[im][:, q:q + 1])
                    if q < 5:
                        nc.sync.dma_start(out=out_o[im, 128 * q:128 * (q + 1), :],
                                          in_=osb)
                    else:
                        nc.sync.dma_start(out=out_o[im, 640:J, :],
                                          in_=osb[0:J - 640, :])
    nc.finalize()
    return nc


def kernel(heat: np.ndarray, feat: np.ndarray, reg_targets: np.ndarray) -> np.ndarray:
    heat = np.ascontiguousarray(heat, dtype=np.float32)
    feat = np.ascontiguousarray(feat, dtype=np.float32).reshape(B, CF, HW)
    regs = np.ascontiguousarray(reg_targets, dtype=np.float32)

    nc = build_program()
    in_maps = [
        {"heat": heat[c * IPC:(c + 1) * IPC],
         "feat": feat[c * IPC:(c + 1) * IPC],
         "regs": regs}
        for c in range(NCORES)
    ]
    res = run_bass_kernel_spmd(nc, in_maps, list(range(NCORES)))
    out = np.concatenate([np.asarray(r["out"]) for r in res.results], axis=0)
    return out.reshape(B, J, CF)


if __name__ == "__main__":
    import ref_numpy as RN
    inputs = RN.get_inputs()
    exp = RN.get_expected(inputs)
    got = kernel(inputs["heat"], inputs["feat"], inputs["reg_targets"])
    err = np.abs(got - exp).max() / np.abs(exp).max()
    print("Relative error:", err)
